# revision 87
# baseline (speedup 1.0000x reference)
"""Trainium2 Bass kernel for nn_CrossAttention (no-softmax cross attention + residual + LayerNorm).

Reference computes:
    q = node @ W_q.T ; k = obs @ W_k.T ; v = obs @ W_v.T
    out = (q @ k.T) @ v ;  result = LayerNorm(out + node) * gamma + beta

Since there is no softmax, matmul associativity gives
    out + node = node @ (W_q.T @ W_k @ (obs.T @ obs) @ W_v.T + I) = node @ W_tot
which cuts 237 GFLOP to ~29 GFLOP (the headroom-8 reassociation).

Strategy (8 NeuronCores, SPMD):
  - Shard node rows 8 ways (6250 rows/core); replicate obs + weights.
  - Prelude builds W_tot on-chip with a short obs-dependent tail:
      A1T = W_k.T @ W_q   (both natural layouts - no transpose, runs during obs DMA)
      G   = obs.T @ obs   (Gram contracts over partition dim - no transpose;
                           accumulated per obs DMA chunk as data streams in)
      T1  = G @ W_v.T ;  W_tot = A1 @ T1 + I
    The +I is folded by adding the identity to the copied W_tot diagonal
    blocks on DVE (not via an extra matmul - the prelude-exit chain is
    PE-serial, so that saves 4x512-free matmuls on the critical path).  The
    exit chain's PSUM->SBUF bounces alternate between ACT and DVE so the
    exit isn't serialized on one engine.
  - The per-core program is DMA-bound: 29.6 MB of traffic / 360 B/ns of
    modeled DMA bandwidth = 82.7us on the shared DMA-engine device, so the
    schedule exists to keep that device saturated end-to-end.
  - Loads-first schedule: ALL 49 node row-tile loads are emitted before the
    compute pipeline (the Tile scheduler orders the shared DMA device by
    program priority), so loads stream back-to-back and the store backlog
    drains the device right to the end - no tail starvation waiting on
    compute.  Stores ride the SP queue, which is idle once loads finish.
  - Pipeline per group of 2 tiles: PE transpose (node.T) -> PSUM->SBUF bounce
    (alternating ACT/DVE) -> 4 accumulating bf16 matmuls -> LayerNorm
    (bn_stats/bn_aggr on DVE, batched sqrt on ACT, normalize on ACT) -> store.
    Transposes lead matmuls by one group so the PE queue never head-blocks on
    a bounce copy.
  - Tail shaping: transposes switch to f32r (1.5 vs 2.0 cyc/row) from tile 30
    where PE becomes the critical engine; groups shrink to single tiles from
    tile 44 and LN normalize alternates ACT/DVE there, shortening the final
    mm -> LN -> store chain that bounds the end of the run.
  - _legalize_matmult_waits(): the loads-first schedule leaves some Matmults
    with 2+ sem waits (HW allows 1 on a matmult + 1 on its Ldweights);
    excess waits are hoisted onto InstEventSemaphore instructions inserted
    just before them on the in-order PE queue.  Pure-pacing waits (PE
    waiting on a PE-only semaphore, or a DRAM store waiting on PE) are
    dropped outright.
  - ~30 dep-free junk transposes at t~0 warm the PE p-state ramp; keeping PE
    *continuously* busy holds the fast clock (less PE work measurably loses
    to p-state resets, which is why transposes stay f32 in the DMA-paced
    phase).
  - fp32r (rounded-fp32 single-pass PE mode) measured at l2 rel-err ~1.5e-4 vs
    fp64, essentially identical to this HW's fp32 matmul, at 4x the speed.
  - Cost-model sim: 89.3us vs 82.7us DMA busy + 2.0us head + 1.5us drain
    (irreducible ~86.2us); session start was 98.0us, original baseline 107.9us.
    The residual ~3us is a three-way equilibrium: PE transpose pace gates the
    final loads, ACT/DVE balance gates LN+bounce copies, and both sides of
    that deficit surface as DMA idle wherever the load tail is placed.
"""

import numpy as np
from contextlib import ExitStack

import concourse.bacc as bacc
import concourse.bass as bass
import concourse.tile as tile
import concourse.mybir as mybir
import concourse.masks as masks

F32 = mybir.dt.float32
F32R = mybir.dt.float32r
BF16 = mybir.dt.bfloat16
AF = mybir.ActivationFunctionType
ALU = mybir.AluOpType

N_TOT, M, E, O = 50000, 2048, 512, 256
N_CORES = 8
NP = N_TOT // N_CORES          # 6250 rows per core
EPS = 1e-6
P = 128
KE = E // P                    # 4 contraction tiles over E
KO = O // P                    # 2 contraction tiles over O
MT = M // P                    # 16 obs row tiles
NT = (NP + P - 1) // P         # 49 node row tiles per core
LAST = NP - (NT - 1) * P       # 106 rows in the last tile

# tuning knobs (sim-swept)
KNOBS = dict(node_bufs=14, ndt_bufs=28, out_bufs=24, stat_bufs=8,
             pst_bufs=3, acc_bufs=5, group=2, store_engine="sync",
             norm_alt=False, obs_chunks=8, prelude_dma="sync", store_lag=2,
             tr_mode="f32", obs_first=False, pe_warm=30, store_grp=False,
             tr_lead=1, sched="loadfirst", tr_tail_from=30,
             tr_tail_mode="f32r", norm_alt_from=42, pre_copy_alt=True,
             tail_grp1_from=44)


def _build(apply_affine: bool, knob_overrides: dict | None = None):
    if knob_overrides:
        KNOBS.update(knob_overrides)
    nc = bacc.Bacc("TRN2", target_bir_lowering=False, debug=False,
                   num_devices=N_CORES)
    node = nc.dram_tensor("node", [NP, E], F32, kind="ExternalInput")
    obs = nc.dram_tensor("obs", [M, O], F32, kind="ExternalInput")
    wq = nc.dram_tensor("wq", [E, E], F32, kind="ExternalInput")
    wk = nc.dram_tensor("wk", [E, O], F32, kind="ExternalInput")
    wv = nc.dram_tensor("wv", [E, O], F32, kind="ExternalInput")
    if apply_affine:
        gam = nc.dram_tensor("gam", [1, E], F32, kind="ExternalInput")
        bet = nc.dram_tensor("bet", [1, E], F32, kind="ExternalInput")
    out = nc.dram_tensor("out", [NP, E], F32, kind="ExternalOutput")

    with tile.TileContext(nc) as tc, ExitStack() as ctx:
        const = ctx.enter_context(tc.tile_pool(name="const", bufs=1))
        wtot_pool = ctx.enter_context(tc.tile_pool(name="wtotp", bufs=1))

        ident = const.tile([P, P], F32)
        masks.make_identity(nc, ident[:])
        if KNOBS["tr_mode"] == "bf16" or \
                KNOBS.get("tr_tail_mode") == "bf16":
            ident_b = const.tile([P, P], BF16)
            nc.vector.tensor_copy(ident_b[:], ident[:])
        if KNOBS["tr_mode"] == "f32r" or \
                KNOBS.get("tr_tail_mode") == "f32r":
            ident_fr = const.tile([P, P], F32R)
            nc.vector.tensor_copy(ident_fr[:], ident[:])
        eps_t = const.tile([P, 1], F32)
        nc.gpsimd.memset(eps_t[:], EPS)

        wtot = wtot_pool.tile([P, KE, E], BF16)   # W_tot, k-tiled over rows
        if apply_affine:
            gbc = const.tile([P, E], F32)         # gamma broadcast
            bbc = const.tile([P, E], F32)         # beta broadcast

        # Main-loop SBUF pools are opened BEFORE the prelude scratch pool so
        # they get fresh addresses: otherwise the bump allocator reuses the
        # prelude ranges and Tile serializes the first node loads behind the
        # entire prelude (a ~12us false dependency on the DMA dispatch queue).
        # Buffer depths matter mostly at the load tail: nd slots are freed by
        # the transposes (PE-paced), and node_bufs sets how far the load
        # stream can run ahead of PE before the DMA device idles.
        node_pool = ctx.enter_context(
            tc.tile_pool(name="nodep", bufs=KNOBS["node_bufs"]))
        ndT_pool = ctx.enter_context(
            tc.tile_pool(name="ndtp", bufs=KNOBS["ndt_bufs"]))
        out_pool = ctx.enter_context(
            tc.tile_pool(name="outp", bufs=KNOBS["out_bufs"]))
        stat_pool = ctx.enter_context(
            tc.tile_pool(name="statp", bufs=KNOBS["stat_bufs"]))

        GRP = KNOBS["group"]
        node_ap = node.ap()

        def load_group(t0g, gmax=None):
            """Emit the DMA load for one group; returns state for stage_tr3."""
            g = min(gmax or GRP, NT - t0g)
            r0 = t0g * P
            full = (t0g + g < NT) or (LAST == P)
            rows = g * P if full else (g - 1) * P + LAST
            # tail groups transposed in f32r need an f32r-typed DMA dest (the
            # BIR verifier requires f32r matmul inputs to be produced as f32r)
            ttf = KNOBS.get("tr_tail_from")
            as_f32r = (ttf is not None and t0g >= ttf
                       and KNOBS.get("tr_tail_mode", "f32r") == "f32r")
            nd_dt = F32R if as_f32r else F32
            if gmax == 1 and KNOBS.get("nd1_bufs"):
                # tail singles get a private half-size tag with enough slots
                # that their loads never wait on transpose progress
                nd = node_pool.tile([P, 1, E], nd_dt, tag="nd1",
                                    bufs=KNOBS["nd1_bufs"], name="nd1")
            else:
                nd = node_pool.tile([P, GRP, E], nd_dt, tag="nd")
            # tail loads are watermark-gated (nd slot reuse waits on PE
            # transposes); dispatching them from another queue keeps their
            # waits from head-blocking the stores queued behind them on SP
            tlf = KNOBS.get("tail_load_from")
            ld = nc.sync.dma_start
            if tlf is not None and t0g >= tlf:
                ld = {"scalar": nc.scalar.dma_start,
                      "gpsimd": nc.gpsimd.dma_start}[
                    KNOBS.get("tail_load_engine", "gpsimd")]
            if full and g == GRP:
                src = node_ap[r0:r0 + g * P, :].rearrange(
                    "(b p) e -> p b e", p=P)
                ld(nd[:], src.bitcast(F32R) if as_f32r else src)
            else:
                for j in range(g):
                    rn_j = min(P, rows - j * P)
                    srcj = node_ap[r0 + j * P:r0 + j * P + rn_j, :]
                    ld(nd[:rn_j, j, :],
                       srcj.bitcast(F32R) if as_f32r else srcj)
            return (t0g, g, nd, rows)

        pre_items = []

        # -------- prelude: W_tot = (W_q.T @ W_k) @ (G @ W_v.T) + I -----------
        with ExitStack() as pctx:
            sc = pctx.enter_context(tc.tile_pool(name="presb", bufs=1))
            pps = pctx.enter_context(
                tc.tile_pool(name="preps", bufs=4, space="PSUM"))

            # PE warmup: dep-free transposes keep the tensor engine busy from
            # t~0 so the first real matmuls dispatch at full p-state (the cost
            # ramp resets after idle gaps). Input is a DVE-memset junk tile
            # (gpsimd memset has a ~1.2us Q7 launch; DVE is ready sooner);
            # the output is scratch and never read.
            if KNOBS["pe_warm"]:
                junk = sc.tile([P, P], F32)
                nc.vector.memset(junk[:], 0.0)
                warm_ps = pps.tile([P, P], F32, tag="warm")
                for _w in range(KNOBS["pe_warm"]):
                    nc.tensor.transpose(warm_ps[:], junk[:], junk[:])

            if KNOBS.get("i_fold", "dve") == "dve":
                # +I folded by adding the identity to the copied W_tot blocks
                # on DVE - saves a 512-free matmul per block on the PE-serial
                # prelude-exit chain (and the 1 MB zsh scratch)
                ident_bw = sc.tile([P, P], BF16)
                nc.vector.tensor_copy(ident_bw[:], ident[:])
            else:
                # identity in f32r + shifted identity block for the +I fold
                ident_r = sc.tile([P, P], F32R)
                nc.vector.tensor_copy(ident_r[:], ident[:])
                zsh = sc.tile([P, 2 * KE * P], F32)  # I at cols [512:640)
                nc.gpsimd.memset(zsh[:], 0.0)
                nc.gpsimd.affine_select(
                    out=zsh[:, KE * P:(KE + 1) * P],
                    in_=zsh[:, KE * P:(KE + 1) * P],
                    compare_op=ALU.not_equal, fill=1.0, base=0,
                    pattern=[[-1, P]], channel_multiplier=1)
                zsh_r = sc.tile([P, 2 * KE * P], F32R)
                nc.vector.tensor_copy(zsh_r[:], zsh[:])

            pre_dma = {"vector": nc.vector.dma_start,
                       "scalar": nc.scalar.dma_start,
                       "gpsimd": nc.gpsimd.dma_start,
                       "sync": nc.sync.dma_start}[KNOBS["prelude_dma"]]

            # declare prelude input tiles
            wk_sb = sc.tile([P, KE, O], F32R)
            wq_sb = sc.tile([P, KE, E], F32R)
            wv_sb = sc.tile([P, KE, O], F32)
            n_chunks = KNOBS["obs_chunks"]
            assert MT % n_chunks == 0, \
                f"obs_chunks={n_chunks} must divide MT={MT}"
            cm = MT // n_chunks
            obs_rot = KNOBS.get("obs_rot")   # rotating chunk window (0=full)
            if obs_rot:
                obs_tiles = []
            else:
                obs_sb = sc.tile([P, MT, O], F32R)
            obs_re = obs.ap().rearrange("(t p) o -> p t o", p=P).bitcast(F32R)

            def load_wk():
                pre_dma(wk_sb[:],
                        wk.ap().rearrange("(k p) o -> p k o", p=P).bitcast(F32R))

            def load_wq():
                pre_dma(wq_sb[:],
                        wq.ap().rearrange("(k p) x -> p k x", p=P).bitcast(F32R))

            def load_wv():
                pre_dma(wv_sb[:],
                        wv.ap().rearrange("(k p) o -> p k o", p=P))

            def load_obs():
                # obs streamed in chunks; G accumulates per chunk.  With
                # obs_rot, chunks rotate through a small window instead of a
                # full-obs buffer (2 MB -> cm*obs_rot tiles), freeing SBUF
                # for more nd bufs; each chunk is consumed by its G matmuls
                # right after landing, so a shallow window never stalls.
                for c in range(n_chunks):
                    if obs_rot:
                        och = sc.tile([P, cm, O], F32R, tag="obsch",
                                      bufs=obs_rot, name=f"obsch{c}")
                        obs_tiles.append(och)
                        pre_dma(och[:], obs_re[:, c * cm:(c + 1) * cm, :])
                    else:
                        pre_dma(obs_sb[:, c * cm:(c + 1) * cm, :],
                                obs_re[:, c * cm:(c + 1) * cm, :])

            order = KNOBS.get("prelude_order")
            if order is None:
                order = "owv" if KNOBS["obs_first"] else "wvo"
            def load_nodes_pre():
                for gi in range(KNOBS.get("pre_node_groups", 0)):
                    pre_items.append(load_group(gi * GRP))

            _loaders = {"k": load_wk, "q": load_wq, "v": load_wv,
                        "o": load_obs, "n": load_nodes_pre}
            _seq = {"wvo": "kqvo", "owv": "okqv",
                    "kqov": "kqov", "kqvo": "kqvo",
                    "kvqo": "kvqo", "okvq": "okvq",
                    "koqv": "koqv", "kovq": "kovq", "okqv": "okqv",
                    "nkqvo": "nkqvo", "knqvo": "knqvo",
                    "kqnvo": "kqnvo", "kqvno": "kqvno"}[order]
            for ch in _seq:
                _loaders[ch]()

            # A1T = W_k.T @ W_q  [256, 512] - no obs dependency
            a1t_sb = sc.tile([P, KO, E], F32R)
            for a in range(KO):
                a1_ps = pps.tile([P, E], F32, tag="pps")
                for k in range(KE):
                    nc.tensor.matmul(
                        a1_ps[:], wk_sb[:, k, a * P:(a + 1) * P], wq_sb[:, k, :],
                        start=(k == 0), stop=(k == KE - 1))
                nc.scalar.copy(a1t_sb[:, a, :], a1_ps[:])

            # W_v.T  [256, 512] via PE transpose - no obs dependency
            wvT_sb = sc.tile([P, KO, E], F32R)
            for b in range(KO):
                t_ps = pps.tile([P, E], F32, tag="pps")
                for j in range(KE):
                    nc.tensor.transpose(
                        t_ps[:, j * P:(j + 1) * P],
                        wv_sb[:, j, b * P:(b + 1) * P], ident[:])
                nc.scalar.copy(wvT_sb[:, b, :], t_ps[:])

            # G = obs.T @ obs  [256, 256], accumulated chunk by chunk.
            # WARNING: do NOT move g_ps to its own PSUM tag (or change the
            # warm_ps bufs): that re-tagging compiled and matched the cost
            # model (89267ns) but produced rel-err 0.16 on hardware - the
            # long-lived G accumulation interleaved with other matmul groups
            # appears sensitive to PSUM bank placement.  Keep the shared
            # "pps" ring layout that hardware-validates.
            g_ps = [pps.tile([P, O], F32, tag="pps", name=f"g_ps{a}")
                    for a in range(KO)]
            for c in range(n_chunks):
                for a in range(KO):
                    for t in range(c * cm, (c + 1) * cm):
                        if obs_rot:
                            lhs = obs_tiles[c][:, t - c * cm,
                                               a * P:(a + 1) * P]
                            rhs = obs_tiles[c][:, t - c * cm, :]
                        else:
                            lhs = obs_sb[:, t, a * P:(a + 1) * P]
                            rhs = obs_sb[:, t, :]
                        nc.tensor.matmul(
                            g_ps[a][:], lhs, rhs,
                            start=(t == 0), stop=(t == MT - 1))
            g_sb = sc.tile([P, KO, O], F32R)
            for a in range(KO):
                (nc.vector.tensor_copy if KNOBS.get("pre_copy_alt") and
                 a % 2 else nc.scalar.copy)(g_sb[:, a, :], g_ps[a][:])

            # T1 = G @ W_v.T  [256, 512]  (G symmetric -> G tiles usable as lhsT)
            t1_sb = sc.tile([P, KO, E], F32R)
            for a in range(KO):
                t1_ps = pps.tile([P, E], F32, tag="pps")
                for b in range(KO):
                    nc.tensor.matmul(
                        t1_ps[:], g_sb[:, b, a * P:(a + 1) * P], wvT_sb[:, b, :],
                        start=(b == 0), stop=(b == KO - 1))
                (nc.vector.tensor_copy if KNOBS.get("pre_copy_alt") and
                 a % 2 == 0 else nc.scalar.copy)(t1_sb[:, a, :], t1_ps[:])

            # W_tot = A1 @ T1 + I  [512, 512]
            dve_fold = KNOBS.get("i_fold", "dve") == "dve"
            if dve_fold and KNOBS.get("wtot_bmajor", False):
                # b-major emission: the first KE matmuls need only T1 block 0,
                # hiding T1 block 1's compute+copy latency behind real PE work
                # on the prelude-exit critical chain
                w_pss = [pps.tile([P, E], F32, tag="pps", name=f"w_ps{x}")
                         for x in range(KE)]
                for b in range(KO):
                    for x in range(KE):
                        nc.tensor.matmul(
                            w_pss[x][:], a1t_sb[:, b, x * P:(x + 1) * P],
                            t1_sb[:, b, :], start=(b == 0),
                            stop=(b == KO - 1), skip_group_check=True)
                for x in range(KE):
                    (nc.vector.tensor_copy if KNOBS.get("pre_copy_alt") and
                     x % 2 else nc.scalar.copy)(wtot[:, x, :], w_pss[x][:])
                    nc.vector.tensor_add(
                        wtot[:, x, x * P:(x + 1) * P],
                        wtot[:, x, x * P:(x + 1) * P], ident_bw[:])
            else:
                for x in range(KE):
                    w_ps = pps.tile([P, E], F32, tag="pps")
                    for b in range(KO):
                        nc.tensor.matmul(
                            w_ps[:], a1t_sb[:, b, x * P:(x + 1) * P],
                            t1_sb[:, b, :], start=(b == 0),
                            stop=(dve_fold and b == KO - 1))
                    if not dve_fold:
                        nc.tensor.matmul(
                            w_ps[:], ident_r[:],
                            zsh_r[:, KE * P - x * P: 2 * KE * P - x * P],
                            start=False, stop=True)
                    (nc.vector.tensor_copy if KNOBS.get("pre_copy_alt") and
                     x % 2 else nc.scalar.copy)(wtot[:, x, :], w_ps[:])
                    if dve_fold:
                        nc.vector.tensor_add(
                            wtot[:, x, x * P:(x + 1) * P],
                            wtot[:, x, x * P:(x + 1) * P], ident_bw[:])

            if apply_affine:
                ones_r = sc.tile([1, P], F32R)
                nc.gpsimd.memset(ones_r[:], 1.0)
                gam_sb = sc.tile([1, E], F32R)
                nc.sync.dma_start(gam_sb[:], gam.ap().bitcast(F32R))
                bet_sb = sc.tile([1, E], F32R)
                nc.sync.dma_start(bet_sb[:], bet.ap().bitcast(F32R))
                for (src, dst) in ((gam_sb, gbc), (bet_sb, bbc)):
                    bc_ps = pps.tile([P, E], F32, tag="pps")
                    nc.tensor.matmul(bc_ps[:], ones_r[:], src[:])
                    nc.scalar.copy(dst[:], bc_ps[:])

        # ---------------- main loop over node row tiles ----------------------
        psT_pool = ctx.enter_context(
            tc.tile_pool(name="pstp", bufs=KNOBS["pst_bufs"], space="PSUM"))
        acc_pool = ctx.enter_context(
            tc.tile_pool(name="accp", bufs=KNOBS["acc_bufs"], space="PSUM"))
        _eng = {"scalar": nc.scalar.dma_start,
                "gpsimd": nc.gpsimd.dma_start,
                "sync": nc.sync.dma_start}
        _st_cnt = [0]

        def store_dma(dst, src, tile_idx=None):
            se = KNOBS["store_engine"]
            pr = KNOBS.get("pool_store_range")
            if pr is not None and tile_idx is not None \
                    and pr[0] <= tile_idx < pr[1]:
                _st_cnt[0] += 1
                return nc.gpsimd.dma_start(dst, src)
            if se == "alt":          # alternate ACT / SP queues
                fn = (nc.scalar.dma_start if _st_cnt[0] % 2 == 0
                      else nc.sync.dma_start)
            elif se == "altg":       # alternate ACT / Pool queues
                fn = (nc.scalar.dma_start if _st_cnt[0] % 2 == 0
                      else nc.gpsimd.dma_start)
            elif se == "sg":         # alternate SP / Pool queues
                fn = (nc.sync.dma_start if _st_cnt[0] % 2 == 0
                      else nc.gpsimd.dma_start)
            elif se == "sga":        # rotate SP / Pool / ACT queues
                fn = (nc.sync.dma_start, nc.gpsimd.dma_start,
                      nc.scalar.dma_start)[_st_cnt[0] % 3]
            else:
                fn = _eng[se]
            _st_cnt[0] += 1
            return fn(dst, src)

        node_ap = node.ap()
        out_ap = out.ap()
        GRP = KNOBS["group"]

        def stage_tr(t0g, g):
            """Loads + PE transposes + PSUM->SBUF copies for one group.
            Returns [(ndT, rn, r0), ...] for stage_acc."""
            r0 = t0g * P
            full = (t0g + g < NT) or (LAST == P)
            rows = g * P if full else (g - 1) * P + LAST
            nd = node_pool.tile([P, GRP, E], F32, tag="nd")
            if full and g == GRP:
                nc.sync.dma_start(
                    nd[:], node_ap[r0:r0 + g * P, :].rearrange(
                        "(b p) e -> p b e", p=P))
            else:
                for j in range(g):
                    rn_j = min(P, rows - j * P)
                    nc.sync.dma_start(nd[:rn_j, j, :],
                                      node_ap[r0 + j * P:r0 + j * P + rn_j, :])
            tr_mode = KNOBS["tr_mode"]
            if tr_mode == "bf16":
                # downcast once per group, then 1.0 cyc/row PE transposes
                ndb = ndT_pool.tile([P, GRP, E], BF16, tag="ndb")
                for j in range(g):
                    rn = min(P, rows - j * P)
                    cv = nc.scalar.copy if (t0g + j) % 2 == 0 \
                        else nc.vector.tensor_copy
                    cv(ndb[:rn, j, :], nd[:rn, j, :])
            ps_dt = {"bf16": BF16, "f32r": F32R, "f32": F32}[tr_mode]
            trs = []
            for j in range(g):
                rn = min(P, rows - j * P)
                psT = psT_pool.tile([P, E], ps_dt, tag="psT")
                for k in range(KE):
                    if tr_mode == "bf16":
                        nc.tensor.transpose(
                            psT[:, k * P:k * P + rn],
                            ndb[:rn, j, k * P:(k + 1) * P], ident_b[:rn, :rn])
                    elif tr_mode == "f32r":
                        nc.tensor.transpose(
                            psT[:, k * P:k * P + rn],
                            nd[:rn, j, k * P:(k + 1) * P].bitcast(F32R),
                            ident_fr[:rn, :rn])
                    else:
                        nc.tensor.transpose(
                            psT[:, k * P:k * P + rn],
                            nd[:rn, j, k * P:(k + 1) * P], ident[:rn, :rn])
                ndT = ndT_pool.tile([P, E], BF16, tag="ndT")
                t = t0g + j
                cp = nc.scalar.copy if t % 2 == 0 else nc.vector.tensor_copy
                ps_src = psT.bitcast(F32) if tr_mode == "f32r" else psT
                if rn == P:
                    cp(ndT[:], ps_src[:])
                else:
                    for k in range(KE):
                        cp(ndT[:, k * P:k * P + rn],
                           ps_src[:, k * P:k * P + rn])
                trs.append((ndT, rn, r0 + j * P))
            return trs

        def stage_acc(trs):
            """Accumulating matmuls for one group. Returns [(acc, rn, r0)]."""
            accs = []
            for ndT, rn, r0 in trs:
                acc = acc_pool.tile([P, E], F32, tag="acc")
                for k in range(KE):
                    nc.tensor.matmul(
                        acc[:rn, :], ndT[:, k * P:k * P + rn], wtot[:, k, :],
                        start=(k == 0), stop=(k == KE - 1))
                accs.append((acc, rn, r0))
            return accs

        def stage_mm(t0g, g):
            """Original per-tile interleaved emission: tr a, copy a, mm a,
            tr b, copy b, mm b — measurably better for the PE pipeline than
            batching all transposes before all matmuls."""
            r0 = t0g * P
            full = (t0g + g < NT) or (LAST == P)
            rows = g * P if full else (g - 1) * P + LAST
            nd = node_pool.tile([P, GRP, E], F32, tag="nd")
            if full and g == GRP:
                nc.sync.dma_start(
                    nd[:], node_ap[r0:r0 + g * P, :].rearrange(
                        "(b p) e -> p b e", p=P))
            else:
                for j in range(g):
                    rn_j = min(P, rows - j * P)
                    nc.sync.dma_start(nd[:rn_j, j, :],
                                      node_ap[r0 + j * P:r0 + j * P + rn_j, :])
            tr_mode = KNOBS["tr_mode"]
            ps_dt = {"bf16": BF16, "f32r": F32R, "f32": F32}[tr_mode]
            accs = []
            for j in range(g):
                rn = min(P, rows - j * P)
                psT = psT_pool.tile([P, E], ps_dt, tag="psT")
                for k in range(KE):
                    nc.tensor.transpose(
                        psT[:, k * P:k * P + rn],
                        nd[:rn, j, k * P:(k + 1) * P], ident[:rn, :rn])
                ndT = ndT_pool.tile([P, E], BF16, tag="ndT")
                t = t0g + j
                cc = KNOBS.get("copy_chunks", 1)
                if rn == P and cc == 1:
                    cp = (nc.scalar.copy if t % 2 == 0
                          else nc.vector.tensor_copy)
                    cp(ndT[:], psT[:])
                elif rn == P:
                    # chunked bounce copy: mm k can start after chunk k lands
                    w = E // cc
                    for c in range(cc):
                        cp = (nc.scalar.copy if (t + c) % 2 == 0
                              else nc.vector.tensor_copy)
                        cp(ndT[:, c * w:(c + 1) * w], psT[:, c * w:(c + 1) * w])
                else:
                    cp = (nc.scalar.copy if t % 2 == 0
                          else nc.vector.tensor_copy)
                    for k in range(KE):
                        cp(ndT[:, k * P:k * P + rn], psT[:, k * P:k * P + rn])
                acc = acc_pool.tile([P, E], F32, tag="acc")
                for k in range(KE):
                    nc.tensor.matmul(
                        acc[:rn, :], ndT[:, k * P:k * P + rn], wtot[:, k, :],
                        start=(k == 0), stop=(k == KE - 1))
                accs.append((acc, rn, r0 + j * P))
            return accs

        def stage_ln(accs):
            """LayerNorm for a group; sqrt/recip batched across the group.
            Returns [(ot, rn, r0), ...] for the store stage."""
            g = len(accs)
            mv = stat_pool.tile([P, g, 2], F32, tag="mv")
            for j, (acc, rn, _) in enumerate(accs):
                bn6 = stat_pool.tile([P, 6], F32, tag="bn6")
                nc.vector.bn_stats(bn6[:rn], acc[:rn, :])
                nc.vector.bn_aggr(mv[:rn, j, :], bn6[:rn])
            rnmax = max(rn for _, rn, _ in accs)
            std = stat_pool.tile([P, g], F32, tag="std")
            nc.scalar.activation(std[:rnmax], mv[:rnmax, :, 1], AF.Sqrt,
                                 bias=eps_t[:rnmax], scale=1.0)
            rstd = stat_pool.tile([P, g], F32, tag="rstd")
            nc.vector.reciprocal(rstd[:rnmax], std[:rnmax])
            grp_store = KNOBS["store_grp"]
            otg = (out_pool.tile([P, g, E], F32, tag="otg", name="otg")
                   if grp_store else None)
            outs = []
            for j, (acc, rn, r0) in enumerate(accs):
                nmr = stat_pool.tile([P, 1], F32, tag="nmr")  # -mean * rstd
                nc.vector.tensor_scalar(nmr[:rn], mv[:rn, j, 0:1],
                                        rstd[:rn, j:j + 1], -1.0,
                                        ALU.mult, ALU.mult)
                if grp_store:
                    ot, ot_ap = None, otg[:rn, j, :]
                else:
                    ot = out_pool.tile([P, E], F32, tag="ot")
                    ot_ap = ot[:rn, :]
                naf = KNOBS.get("norm_alt_from")
                npf = KNOBS.get("norm_pool_from")
                tix = r0 // P
                if npf is not None and tix >= npf and tix % 3 == 2:
                    nc.gpsimd.tensor_scalar(ot_ap, acc[:rn, :],
                                            rstd[:rn, j:j + 1], nmr[:rn],
                                            ALU.mult, ALU.add)
                elif (KNOBS["norm_alt"] or (naf is not None and tix >= naf)) \
                        and tix % 2 == 1:
                    nc.vector.tensor_scalar(ot_ap, acc[:rn, :],
                                            rstd[:rn, j:j + 1], nmr[:rn],
                                            ALU.mult, ALU.add)
                else:
                    nc.scalar.activation(ot_ap, acc[:rn, :], AF.Identity,
                                         bias=nmr[:rn], scale=rstd[:rn, j:j + 1])
                if apply_affine:
                    nc.vector.tensor_mul(ot_ap, ot_ap, gbc[:rn])
                    nc.vector.tensor_add(ot_ap, ot_ap, bbc[:rn])
                outs.append((ot, rn, r0))
            return (otg, outs)

        def stage_store(packed):
            otg, outs = packed
            rows = sum(rn for _, rn, _ in outs)
            g = len(outs)
            if otg is not None and g > 1 and rows == g * P:
                # single grouped DMA: one dispatch + one HWDGE slot per group
                r0 = outs[0][2]
                store_dma(
                    out_ap[r0:r0 + g * P, :].rearrange("(b p) e -> p b e", p=P),
                    otg[:, :, :])
                return
            for j, (ot, rn, r0) in enumerate(outs):
                src = otg[:rn, j, :] if otg is not None else ot[:rn, :]
                store_dma(out_ap[r0:r0 + rn, :], src, tile_idx=r0 // P)


        def stage_tr3(item):
            """PE transposes + PSUM->SBUF copies for one pre-loaded group."""
            t0g, g, nd, rows = item
            tr_mode = KNOBS["tr_mode"]
            ttf = KNOBS.get("tr_tail_from")
            if ttf is not None and t0g >= ttf:
                tr_mode = KNOBS.get("tr_tail_mode", "f32r")
            ps_dt = {"bf16": BF16, "f32r": F32R, "f32": F32}[tr_mode]
            if tr_mode == "bf16":
                ndb = ndT_pool.tile([P, GRP, E], BF16, tag="ndb",
                                    bufs=KNOBS.get("ndb_bufs", 4))
                for j in range(g):
                    rn = min(P, rows - j * P)
                    nde = KNOBS.get("ndb_engine")
                    if nde == "gpsimd":
                        # SBUF->SBUF downcast is legal on the (idle) Pool
                        # engine, unlike the PSUM-touching bounce copies
                        cv = nc.gpsimd.tensor_copy
                    else:
                        cv = nc.scalar.copy if (t0g + j) % 2 == 0 \
                            else nc.vector.tensor_copy
                    cv(ndb[:rn, j, :], nd[:rn, j, :])
            trs = []
            for j in range(g):
                rn = min(P, rows - j * P)
                psT = psT_pool.tile([P, E], ps_dt, tag="psT")
                for k in range(KE):
                    if tr_mode == "f32r":
                        nc.tensor.transpose(
                            psT[:, k * P:k * P + rn],
                            nd[:rn, j, k * P:(k + 1) * P],
                            ident_fr[:rn, :rn])
                    elif tr_mode == "bf16":
                        nc.tensor.transpose(
                            psT[:, k * P:k * P + rn],
                            ndb[:rn, j, k * P:(k + 1) * P], ident_b[:rn, :rn])
                    else:
                        nc.tensor.transpose(
                            psT[:, k * P:k * P + rn],
                            nd[:rn, j, k * P:(k + 1) * P], ident[:rn, :rn])
                ndT = ndT_pool.tile([P, E], BF16, tag="ndT")
                t = t0g + j
                # NOTE: gpsimd cannot be used for these copies - the source
                # psT is PSUM and GPSIMD instructions cannot access PSUM
                # (BIR verifier rejects it; the cost model doesn't know).
                if True:
                    par = t % 2
                    cpf = KNOBS.get("copy_flip_from")
                    if cpf is not None and t >= cpf:
                        par ^= 1
                    if t < KNOBS.get("early_copy_dve_until", 0):
                        par = 1      # keep ACT free for the W_tot exit chain
                    cp = nc.scalar.copy if par == 0 else nc.vector.tensor_copy
                ps_src = psT.bitcast(F32) if tr_mode == "f32r" else psT
                if rn == P:
                    cp(ndT[:], ps_src[:])
                else:
                    for k in range(KE):
                        cp(ndT[:, k * P:k * P + rn],
                           ps_src[:, k * P:k * P + rn])
                trs.append((ndT, rn, t0g * P + j * P))
            return trs

        # software pipeline: tr(g+lead) runs ahead of acc(g) so the PSUM->SBUF
        # copy lands before the matmul's Ldweights reaches the PE queue head;
        # LN one group behind acc; stores lag so their data-ready waits never
        # block later dispatches on the store queue.
        LAG = KNOBS.get("store_lag", 1)
        TRL = KNOBS.get("tr_lead", 0)
        if KNOBS["sched"] == "loadfirst":
            # All node loads emitted first: the Tile scheduler (priority =
            # program order) then serves every load before any store on the
            # shared DMA engines, so the DMA device never starves waiting on
            # compute in the tail.  Stores ride the SP queue, which is idle
            # once loads finish.  Transposes lead matmuls by TRL groups so the
            # PE queue never head-blocks on a PSUM->SBUF copy.
            from collections import deque
            grp1_from = KNOBS.get("tail_grp1_from")
            if KNOBS.get("load_lead"):
                # Interleaved emission: the SP queue carries
                # [L0..L(lead), S0,L(lead+1), S1,L(lead+2), ...] so the load
                # stream extends late into the run and ready stores keep the
                # DMA device fed between loads.
                LEADL = KNOBS["load_lead"]
                bounds = []
                t0 = 0
                while t0 < NT:
                    gmax = 1 if (grp1_from is not None and t0 >= grp1_from) \
                        else GRP
                    bounds.append((t0, gmax))
                    t0 += min(gmax, NT - t0)
                pend_loads = deque(
                    load_group(*b) for b in bounds[:LEADL])
                nxt = LEADL
                pend_acc = deque()
                pend_ln = deque()
                pend_st = deque()
                TRL3 = max(1, TRL)
                for i in range(len(bounds)):
                    pend_acc.append(stage_tr3(pend_loads.popleft()))
                    if len(pend_acc) > TRL3:
                        pend_ln.append(stage_acc(pend_acc.popleft()))
                    if len(pend_ln) > 1:
                        pend_st.append(stage_ln(pend_ln.popleft()))
                    if len(pend_st) >= LAG:
                        stage_store(pend_st.popleft())
                    if nxt < len(bounds):
                        pend_loads.append(load_group(*bounds[nxt]))
                        nxt += 1
                while pend_acc:
                    pend_ln.append(stage_acc(pend_acc.popleft()))
                    if len(pend_ln) > 1:
                        pend_st.append(stage_ln(pend_ln.popleft()))
                while pend_ln:
                    pend_st.append(stage_ln(pend_ln.popleft()))
                while pend_st:
                    stage_store(pend_st.popleft())
            items = list(pre_items) if not KNOBS.get("load_lead") else None
            t0 = sum(it[1] for it in pre_items)
            if KNOBS.get("load_lead"):
                pass             # handled above
            elif KNOBS.get("singles_first") and grp1_from is not None \
                    and KNOBS.get("nd1_bufs"):
                # The tail singles' loads are emitted FIRST: their private
                # nd1 slots never recycle, so the data parks in SBUF until
                # their (last-in-pipeline) transposes.  This frees them from
                # the DMA-ring latency chain that otherwise gates the final
                # loads ~2us apart and idles the DMA device.
                k = KNOBS.get("singles_after", 0)
                while t0 < min(grp1_from, k * GRP):
                    items.append(load_group(t0, GRP))
                    t0 += items[-1][1]
                singles = []
                ts = grp1_from
                while ts < NT:
                    singles.append(load_group(ts, 1))
                    ts += 1
                while t0 < grp1_from:
                    items.append(load_group(t0, GRP))
                    t0 += items[-1][1]
                items.extend(singles)
            else:
                while t0 < NT:
                    gmax = 1 if (grp1_from is not None and t0 >= grp1_from) \
                        else GRP
                    items.append(load_group(t0, gmax))
                    t0 += items[-1][1]
            if items is None:
                items = []       # load_lead path already emitted everything
            TRL3 = max(1, TRL)
            trl_tail_from = KNOBS.get("trl_tail_from")
            trl_tail = KNOBS.get("trl_tail", 1)
            pend_acc = deque()
            pend_ln = deque()
            pend_st = deque()
            # Optionally run the tail singles' transposes early, in the
            # mid-phase where PE is input-limited and has slack; the tail
            # then runs matmul+LN only, shrinking the end-of-run compute
            # deficit.  Their ndT tiles park in SBUF until their matmuls.
            EST = KNOBS.get("early_single_tr_at")
            banked_trs = {}
            for item in items:
                t0g = item[0]
                lead = TRL3
                if trl_tail_from is not None and t0g >= trl_tail_from:
                    lead = trl_tail
                if t0g in banked_trs:
                    pend_acc.append(banked_trs.pop(t0g))
                else:
                    pend_acc.append(stage_tr3(item))
                if EST is not None and t0g == EST and grp1_from is not None:
                    for it2 in items:
                        if it2[0] >= grp1_from:
                            banked_trs[it2[0]] = stage_tr3(it2)
                while len(pend_acc) > lead:
                    pend_ln.append(stage_acc(pend_acc.popleft()))
                while len(pend_ln) > 1:
                    pend_st.append(stage_ln(pend_ln.popleft()))
                if len(pend_st) >= LAG:
                    stage_store(pend_st.popleft())
            while pend_acc:
                pend_ln.append(stage_acc(pend_acc.popleft()))
                if len(pend_ln) > 1:
                    pend_st.append(stage_ln(pend_ln.popleft()))
            while pend_ln:
                pend_st.append(stage_ln(pend_ln.popleft()))
            while pend_st:
                stage_store(pend_st.popleft())
        else:
            _emit_interleave(stage_tr, stage_acc, stage_mm, stage_ln,
                             stage_store, TRL, LAG, GRP)

    _legalize_matmult_waits(nc)
    nc.compile()
    return nc


def _legalize_matmult_waits(nc):
    """Split multi-wait Matmults: HW allows 1 sync wait on a matmult (plus 1
    on its Ldweights); the loadfirst schedule's pacing sems can exceed that.
    Hoist ALL waits onto InstEventSemaphore instructions (2 waits each)
    inserted just before the matmult on the same in-order queue.

    Waits on semaphores updated ONLY by same-engine (PE) instructions are
    dropped outright: the PE queue executes in order, so any such update
    precedes this instruction's execution; these waits are scheduler pacing,
    not data dependencies."""
    pe_only_sems = {}
    dma_sems = set()
    for blk in nc.m.functions[0].blocks:
        for inst in blk.instructions:
            si = inst.sync_info
            if si is None:
                continue
            for u in si.on_update:
                key = (str(u.sync_type), u.id)
                if inst.engine != mybir.EngineType.PE:
                    pe_only_sems[key] = False
                else:
                    pe_only_sems.setdefault(key, True)
                if inst.opcode == "DMACopy":
                    dma_sems.add(key)
    for blk in nc.m.functions[0].blocks:
        insts = blk.instructions
        idx = 0
        while idx < len(insts):
            inst = insts[idx]
            si = inst.sync_info
            if si is not None and si.on_wait \
                    and inst.opcode == "DMACopy" \
                    and KNOBS.get("drop_nd1_waits", True):
                # nd1 single loads write private never-reused slots: no WAR
                # hazard exists, so any non-DMA wait is scheduler pacing
                try:
                    is_nd1 = str(inst.outs[0].memref).startswith("nd1")
                except Exception:
                    is_nd1 = False
                if is_nd1:
                    kept = [w for w in si.on_wait
                            if (str(w.sync_type), w.id) in dma_sems]
                    if len(kept) != len(si.on_wait):
                        inst.sync_info = mybir.SyncInfo(
                            on_wait=kept, on_update=si.on_update)
                        si = inst.sync_info
            if si is not None and si.on_wait \
                    and inst.opcode == "DMACopy" \
                    and KNOBS.get("drop_store_pe_waits", True):
                # an output store never depends on PE directly (LN on ACT/DVE
                # produces its data); PE-sem waits on stores are scheduler
                # pacing only
                try:
                    is_store = inst.outs[0].memref == "out"
                except Exception:
                    is_store = False
                if is_store:
                    kept = [w for w in si.on_wait
                            if not pe_only_sems.get(
                                (str(w.sync_type), w.id), False)]
                    if len(kept) != len(si.on_wait):
                        inst.sync_info = mybir.SyncInfo(
                            on_wait=kept, on_update=si.on_update)
                        si = inst.sync_info
            if si is not None and si.on_wait \
                    and inst.engine == mybir.EngineType.PE \
                    and KNOBS.get("drop_pe_waits", True):
                kept = [w for w in si.on_wait
                        if not pe_only_sems.get(
                            (str(w.sync_type), w.id), False)]
                if len(kept) != len(si.on_wait):
                    inst.sync_info = mybir.SyncInfo(on_wait=kept,
                                                    on_update=si.on_update)
                    si = inst.sync_info
            if inst.opcode == "Matmult" and si is not None \
                    and len(si.on_wait) > 1:
                waits = list(si.on_wait)
                # keep a DMA-sem wait on the matmult if present (it is the
                # late-arriving one); relocate the rest
                keep_i = next(
                    (i for i, w in enumerate(waits)
                     if (str(w.sync_type), w.id) in dma_sems),
                    len(waits) - 1)
                keep = [waits.pop(keep_i)]
                inst.sync_info = mybir.SyncInfo(on_wait=keep,
                                                on_update=si.on_update)
                pos = idx
                for c in range(0, len(waits), 2):
                    ev = mybir.InstEventSemaphore(
                        name=nc.get_next_instruction_name(),
                        engine=inst.engine,
                        ins=[], outs=[],
                        sync_info=mybir.SyncInfo(
                            on_wait=waits[c:c + 2], on_update=[]))
                    nc.register_instruction(ev)
                    insts.insert(pos, ev)
                    pos += 1
                idx = pos
            idx += 1


def _emit_interleave(stage_tr, stage_acc, stage_mm, stage_ln, stage_store,
                     TRL, LAG, GRP):
    for _rep in range(KNOBS.get("repeat", 1)):
            from collections import deque
            pend_acc = deque()  # transposed groups awaiting matmuls
            pend_ln = deque()   # acc'd groups awaiting LN emission
            pend_st = deque()   # LN'd groups awaiting store emission
            t = 0
            while t < NT:
                g = min(GRP, NT - t)
                if TRL == 0:
                    pend_ln.append(stage_mm(t, g))
                else:
                    pend_acc.append(stage_tr(t, g))
                    if len(pend_acc) > TRL:
                        pend_ln.append(stage_acc(pend_acc.popleft()))
                if len(pend_ln) > 1:
                    pend_st.append(stage_ln(pend_ln.popleft()))
                if len(pend_st) >= LAG:
                    stage_store(pend_st.popleft())
                t += g
            while pend_acc:
                pend_ln.append(stage_acc(pend_acc.popleft()))
                if len(pend_ln) > 1:
                    pend_st.append(stage_ln(pend_ln.popleft()))
            while pend_ln:
                pend_st.append(stage_ln(pend_ln.popleft()))
            while pend_st:
                stage_store(pend_st.popleft())

    nc.compile()
    return nc


_CACHE: dict = {}


def _get_runner(apply_affine: bool):
    """Build + jit once; returns a dict with jitted runners + io metadata."""
    key = apply_affine
    if key in _CACHE:
        return _CACHE[key]

    import jax
    from jax.sharding import Mesh, PartitionSpec
    from jax.experimental.shard_map import shard_map
    from concourse import bass2jax

    nc = _build(apply_affine)
    bass2jax.install_neuronx_cc_hook()

    partition_name = (nc.partition_id_tensor.name
                      if nc.partition_id_tensor else None)
    in_names, out_names, out_avals, zero_outs = [], [], [], []
    for alloc in nc.m.functions[0].allocations:
        if not isinstance(alloc, mybir.MemoryLocationSet):
            continue
        name = alloc.memorylocations[0].name
        if alloc.kind == "ExternalInput":
            if name != partition_name:
                in_names.append(name)
        elif alloc.kind == "ExternalOutput":
            shape = tuple(alloc.tensor_shape)
            dtype = mybir.dt.np(alloc.dtype)
            out_names.append(name)
            out_avals.append(jax.core.ShapedArray(shape, dtype))
            zero_outs.append(np.zeros(shape, dtype))
    n_params = len(in_names)
    all_names = in_names + out_names
    if partition_name is not None:
        all_names = all_names + [partition_name]
    donate = tuple(range(n_params, n_params + len(out_names)))

    def _body(*args):
        operands = list(args)
        if partition_name is not None:
            operands.append(bass2jax.partition_id_tensor())
        outs = bass2jax._bass_exec_p.bind(
            *operands,
            out_avals=tuple(out_avals),
            in_names=tuple(all_names),
            out_names=tuple(out_names),
            lowering_input_output_aliases=(),
            sim_require_finite=True,
            sim_require_nnan=True,
            nc=nc,
        )
        return tuple(outs)

    devices = jax.devices()[:N_CORES]
    mesh = Mesh(np.asarray(devices), ("core",))
    n_io = n_params + len(out_names)
    mapped = shard_map(_body, mesh=mesh,
                       in_specs=(PartitionSpec("core"),) * n_io,
                       out_specs=(PartitionSpec("core"),) * len(out_names))
    sharded = jax.jit(mapped, donate_argnums=donate, keep_unused=True)
    sharded_t = jax.jit(mapped, keep_unused=True)  # non-donating, reusable args

    shardings = [jax.sharding.NamedSharding(mesh, PartitionSpec("core"))] * n_io

    def put(arrs):
        return [jax.device_put(a, s) for a, s in zip(arrs, shardings)]

    runner = {
        "fn": sharded,
        "fn_t": sharded_t,
        "put": put,
        "in_names": in_names,
        "out_names": out_names,
        "zero_outs": zero_outs,
    }
    _CACHE[key] = runner
    return runner


def _prep_inputs(runner, inputs_np: dict) -> list:
    """Concat per-core inputs along axis 0 (global arrays for shard_map)."""
    concat = []
    for name in runner["in_names"]:
        per_core = inputs_np[name]           # list of 8 per-core arrays
        concat.append(np.concatenate(per_core, axis=0))
    for z in runner["zero_outs"]:
        concat.append(np.zeros((N_CORES * z.shape[0], *z.shape[1:]), z.dtype))
    return concat


def _make_per_core(node_feature, obs_feature, W_q, W_k, W_v, gam, bet,
                   apply_affine):
    f = np.ascontiguousarray
    per = {
        "node": [f(node_feature[c * NP:(c + 1) * NP]) for c in range(N_CORES)],
        "obs": [f(obs_feature)] * N_CORES,
        "wq": [f(W_q)] * N_CORES,
        "wk": [f(W_k)] * N_CORES,
        "wv": [f(W_v)] * N_CORES,
    }
    if apply_affine:
        per["gam"] = [f(gam.reshape(1, E))] * N_CORES
        per["bet"] = [f(bet.reshape(1, E))] * N_CORES
    return per


def kernel(node_feature, obs_feature, W_q, W_k, W_v, ln_gamma, ln_beta):
    node_feature = np.asarray(node_feature, dtype=np.float32)
    obs_feature = np.asarray(obs_feature, dtype=np.float32)
    W_q = np.asarray(W_q, dtype=np.float32)
    W_k = np.asarray(W_k, dtype=np.float32)
    W_v = np.asarray(W_v, dtype=np.float32)
    ln_gamma = np.asarray(ln_gamma, dtype=np.float32)
    ln_beta = np.asarray(ln_beta, dtype=np.float32)

    apply_affine = not (np.all(ln_gamma == 1.0) and np.all(ln_beta == 0.0))
    runner = _get_runner(apply_affine)
    per = _make_per_core(node_feature, obs_feature, W_q, W_k, W_v,
                         ln_gamma, ln_beta, apply_affine)
    args = _prep_inputs(runner, per)
    outs = runner["fn"](*args)
    res = np.asarray(outs[runner["out_names"].index("out")])
    return res.reshape(N_TOT, E)



# revision 89
# speedup vs baseline: 1.0018x; 1.0018x over previous
"""Trainium2 Bass kernel for nn_CrossAttention (no-softmax cross attention + residual + LayerNorm).

Reference computes:
    q = node @ W_q.T ; k = obs @ W_k.T ; v = obs @ W_v.T
    out = (q @ k.T) @ v ;  result = LayerNorm(out + node) * gamma + beta

Since there is no softmax, matmul associativity gives
    out + node = node @ (W_q.T @ W_k @ (obs.T @ obs) @ W_v.T + I) = node @ W_tot
which cuts 237 GFLOP to ~29 GFLOP (the headroom-8 reassociation).

Strategy (8 NeuronCores, SPMD):
  - Shard node rows 8 ways (6250 rows/core); replicate obs + weights.
  - Prelude builds W_tot on-chip with a short obs-dependent tail:
      A1T = W_k.T @ W_q   (both natural layouts - no transpose, runs during obs DMA)
      G   = obs.T @ obs   (Gram contracts over partition dim - no transpose;
                           accumulated per obs DMA chunk as data streams in)
      T1  = G @ W_v.T ;  W_tot = A1 @ T1 + I
    The +I is folded by adding the identity to the copied W_tot diagonal
    blocks on DVE (not via an extra matmul - the prelude-exit chain is
    PE-serial, so that saves 4x512-free matmuls on the critical path).  The
    exit chain's PSUM->SBUF bounces alternate between ACT and DVE so the
    exit isn't serialized on one engine.
  - The per-core program is DMA-bound: 29.6 MB of traffic / 360 B/ns of
    modeled DMA bandwidth = 82.7us on the shared DMA-engine device, so the
    schedule exists to keep that device saturated end-to-end.
  - Loads-first schedule: ALL 49 node row-tile loads are emitted before the
    compute pipeline (the Tile scheduler orders the shared DMA device by
    program priority), so loads stream back-to-back and the store backlog
    drains the device right to the end - no tail starvation waiting on
    compute.  Stores ride the SP queue, which is idle once loads finish.
  - Pipeline per group of 2 tiles: PE transpose (node.T) -> PSUM->SBUF bounce
    (alternating ACT/DVE) -> 4 accumulating bf16 matmuls -> LayerNorm
    (bn_stats/bn_aggr on DVE, batched sqrt on ACT, normalize on ACT) -> store.
    Transposes lead matmuls by one group so the PE queue never head-blocks on
    a bounce copy.
  - Tail shaping: transposes switch to f32r (1.5 vs 2.0 cyc/row) from tile 30
    where PE becomes the critical engine; groups shrink to single tiles from
    tile 44 and LN normalize alternates ACT/DVE there, shortening the final
    mm -> LN -> store chain that bounds the end of the run.
  - _legalize_matmult_waits(): the loads-first schedule leaves some Matmults
    with 2+ sem waits (HW allows 1 on a matmult + 1 on its Ldweights);
    excess waits are hoisted onto InstEventSemaphore instructions inserted
    just before them on the in-order PE queue.  Pure-pacing waits (PE
    waiting on a PE-only semaphore, or a DRAM store waiting on PE) are
    dropped outright.
  - ~30 dep-free junk transposes at t~0 warm the PE p-state ramp; keeping PE
    *continuously* busy holds the fast clock (less PE work measurably loses
    to p-state resets, which is why transposes stay f32 in the DMA-paced
    phase).
  - fp32r (rounded-fp32 single-pass PE mode) measured at l2 rel-err ~1.5e-4 vs
    fp64, essentially identical to this HW's fp32 matmul, at 4x the speed.
  - Cost-model sim: 89.3us vs 82.7us DMA busy + 2.0us head + 1.5us drain
    (irreducible ~86.2us); session start was 98.0us, original baseline 107.9us.
    The residual ~3us is a three-way equilibrium: PE transpose pace gates the
    final loads, ACT/DVE balance gates LN+bounce copies, and both sides of
    that deficit surface as DMA idle wherever the load tail is placed.
"""

import numpy as np
from contextlib import ExitStack

import concourse.bacc as bacc
import concourse.bass as bass
import concourse.tile as tile
import concourse.mybir as mybir
import concourse.masks as masks

F32 = mybir.dt.float32
F32R = mybir.dt.float32r
BF16 = mybir.dt.bfloat16
AF = mybir.ActivationFunctionType
ALU = mybir.AluOpType

N_TOT, M, E, O = 50000, 2048, 512, 256
N_CORES = 8
NP = N_TOT // N_CORES          # 6250 rows per core
EPS = 1e-6
P = 128
KE = E // P                    # 4 contraction tiles over E
KO = O // P                    # 2 contraction tiles over O
MT = M // P                    # 16 obs row tiles
NT = (NP + P - 1) // P         # 49 node row tiles per core
LAST = NP - (NT - 1) * P       # 106 rows in the last tile

# tuning knobs (sim-swept)
KNOBS = dict(node_bufs=14, ndt_bufs=28, out_bufs=24, stat_bufs=8,
             pst_bufs=3, acc_bufs=5, group=2, store_engine="sync",
             norm_alt=False, obs_chunks=8, prelude_dma="sync", store_lag=2,
             tr_mode="f32", obs_first=False, pe_warm=30, store_grp=False,
             tr_lead=1, sched="loadfirst", tr_tail_from=30,
             tr_tail_mode="f32r", norm_alt_from=42, pre_copy_alt=True,
             tail_grp1_from=44, nmr_pool_from=22)


def _build(apply_affine: bool, knob_overrides: dict | None = None):
    if knob_overrides:
        KNOBS.update(knob_overrides)
    nc = bacc.Bacc("TRN2", target_bir_lowering=False, debug=False,
                   num_devices=N_CORES)
    node = nc.dram_tensor("node", [NP, E], F32, kind="ExternalInput")
    obs = nc.dram_tensor("obs", [M, O], F32, kind="ExternalInput")
    wq = nc.dram_tensor("wq", [E, E], F32, kind="ExternalInput")
    wk = nc.dram_tensor("wk", [E, O], F32, kind="ExternalInput")
    wv = nc.dram_tensor("wv", [E, O], F32, kind="ExternalInput")
    if apply_affine:
        gam = nc.dram_tensor("gam", [1, E], F32, kind="ExternalInput")
        bet = nc.dram_tensor("bet", [1, E], F32, kind="ExternalInput")
    out = nc.dram_tensor("out", [NP, E], F32, kind="ExternalOutput")

    with tile.TileContext(nc) as tc, ExitStack() as ctx:
        const = ctx.enter_context(tc.tile_pool(name="const", bufs=1))
        wtot_pool = ctx.enter_context(tc.tile_pool(name="wtotp", bufs=1))

        ident = const.tile([P, P], F32)
        masks.make_identity(nc, ident[:])
        if KNOBS["tr_mode"] == "bf16" or \
                KNOBS.get("tr_tail_mode") == "bf16":
            ident_b = const.tile([P, P], BF16)
            nc.vector.tensor_copy(ident_b[:], ident[:])
        if KNOBS["tr_mode"] == "f32r" or \
                KNOBS.get("tr_tail_mode") == "f32r":
            ident_fr = const.tile([P, P], F32R)
            nc.vector.tensor_copy(ident_fr[:], ident[:])
        eps_t = const.tile([P, 1], F32)
        nc.gpsimd.memset(eps_t[:], EPS)

        wtot = wtot_pool.tile([P, KE, E], BF16)   # W_tot, k-tiled over rows
        if apply_affine:
            gbc = const.tile([P, E], F32)         # gamma broadcast
            bbc = const.tile([P, E], F32)         # beta broadcast

        # Main-loop SBUF pools are opened BEFORE the prelude scratch pool so
        # they get fresh addresses: otherwise the bump allocator reuses the
        # prelude ranges and Tile serializes the first node loads behind the
        # entire prelude (a ~12us false dependency on the DMA dispatch queue).
        # Buffer depths matter mostly at the load tail: nd slots are freed by
        # the transposes (PE-paced), and node_bufs sets how far the load
        # stream can run ahead of PE before the DMA device idles.
        node_pool = ctx.enter_context(
            tc.tile_pool(name="nodep", bufs=KNOBS["node_bufs"]))
        ndT_pool = ctx.enter_context(
            tc.tile_pool(name="ndtp", bufs=KNOBS["ndt_bufs"]))
        out_pool = ctx.enter_context(
            tc.tile_pool(name="outp", bufs=KNOBS["out_bufs"]))
        stat_pool = ctx.enter_context(
            tc.tile_pool(name="statp", bufs=KNOBS["stat_bufs"]))

        GRP = KNOBS["group"]
        node_ap = node.ap()

        def load_group(t0g, gmax=None):
            """Emit the DMA load for one group; returns state for stage_tr3."""
            g = min(gmax or GRP, NT - t0g)
            r0 = t0g * P
            full = (t0g + g < NT) or (LAST == P)
            rows = g * P if full else (g - 1) * P + LAST
            # tail groups transposed in f32r need an f32r-typed DMA dest (the
            # BIR verifier requires f32r matmul inputs to be produced as f32r)
            ttf = KNOBS.get("tr_tail_from")
            as_f32r = (ttf is not None and t0g >= ttf
                       and KNOBS.get("tr_tail_mode", "f32r") == "f32r")
            nd_dt = F32R if as_f32r else F32
            if gmax == 1 and KNOBS.get("nd1_bufs"):
                # tail singles get a private half-size tag with enough slots
                # that their loads never wait on transpose progress
                nd = node_pool.tile([P, 1, E], nd_dt, tag="nd1",
                                    bufs=KNOBS["nd1_bufs"], name="nd1")
            else:
                nd = node_pool.tile([P, GRP, E], nd_dt, tag="nd")
            # tail loads are watermark-gated (nd slot reuse waits on PE
            # transposes); dispatching them from another queue keeps their
            # waits from head-blocking the stores queued behind them on SP
            tlf = KNOBS.get("tail_load_from")
            ld = nc.sync.dma_start
            if tlf is not None and t0g >= tlf:
                ld = {"scalar": nc.scalar.dma_start,
                      "gpsimd": nc.gpsimd.dma_start}[
                    KNOBS.get("tail_load_engine", "gpsimd")]
            if full and g == GRP:
                src = node_ap[r0:r0 + g * P, :].rearrange(
                    "(b p) e -> p b e", p=P)
                ld(nd[:], src.bitcast(F32R) if as_f32r else src)
            else:
                for j in range(g):
                    rn_j = min(P, rows - j * P)
                    srcj = node_ap[r0 + j * P:r0 + j * P + rn_j, :]
                    ld(nd[:rn_j, j, :],
                       srcj.bitcast(F32R) if as_f32r else srcj)
            return (t0g, g, nd, rows)

        pre_items = []

        # -------- prelude: W_tot = (W_q.T @ W_k) @ (G @ W_v.T) + I -----------
        with ExitStack() as pctx:
            sc = pctx.enter_context(tc.tile_pool(name="presb", bufs=1))
            pps = pctx.enter_context(
                tc.tile_pool(name="preps", bufs=4, space="PSUM"))

            # PE warmup: dep-free transposes keep the tensor engine busy from
            # t~0 so the first real matmuls dispatch at full p-state (the cost
            # ramp resets after idle gaps). Input is a DVE-memset junk tile
            # (gpsimd memset has a ~1.2us Q7 launch; DVE is ready sooner);
            # the output is scratch and never read.
            if KNOBS["pe_warm"]:
                junk = sc.tile([P, P], F32)
                nc.vector.memset(junk[:], 0.0)
                warm_ps = pps.tile([P, P], F32, tag="warm")
                for _w in range(KNOBS["pe_warm"]):
                    nc.tensor.transpose(warm_ps[:], junk[:], junk[:])

            if KNOBS.get("i_fold", "dve") == "dve":
                # +I folded by adding the identity to the copied W_tot blocks
                # on DVE - saves a 512-free matmul per block on the PE-serial
                # prelude-exit chain (and the 1 MB zsh scratch)
                ident_bw = sc.tile([P, P], BF16)
                nc.vector.tensor_copy(ident_bw[:], ident[:])
            else:
                # identity in f32r + shifted identity block for the +I fold
                ident_r = sc.tile([P, P], F32R)
                nc.vector.tensor_copy(ident_r[:], ident[:])
                zsh = sc.tile([P, 2 * KE * P], F32)  # I at cols [512:640)
                nc.gpsimd.memset(zsh[:], 0.0)
                nc.gpsimd.affine_select(
                    out=zsh[:, KE * P:(KE + 1) * P],
                    in_=zsh[:, KE * P:(KE + 1) * P],
                    compare_op=ALU.not_equal, fill=1.0, base=0,
                    pattern=[[-1, P]], channel_multiplier=1)
                zsh_r = sc.tile([P, 2 * KE * P], F32R)
                nc.vector.tensor_copy(zsh_r[:], zsh[:])

            pre_dma = {"vector": nc.vector.dma_start,
                       "scalar": nc.scalar.dma_start,
                       "gpsimd": nc.gpsimd.dma_start,
                       "sync": nc.sync.dma_start}[KNOBS["prelude_dma"]]

            # declare prelude input tiles
            wk_sb = sc.tile([P, KE, O], F32R)
            wq_sb = sc.tile([P, KE, E], F32R)
            wv_sb = sc.tile([P, KE, O], F32)
            n_chunks = KNOBS["obs_chunks"]
            assert MT % n_chunks == 0, \
                f"obs_chunks={n_chunks} must divide MT={MT}"
            cm = MT // n_chunks
            obs_rot = KNOBS.get("obs_rot")   # rotating chunk window (0=full)
            if obs_rot:
                obs_tiles = []
            else:
                obs_sb = sc.tile([P, MT, O], F32R)
            obs_re = obs.ap().rearrange("(t p) o -> p t o", p=P).bitcast(F32R)

            def load_wk():
                pre_dma(wk_sb[:],
                        wk.ap().rearrange("(k p) o -> p k o", p=P).bitcast(F32R))

            def load_wq():
                pre_dma(wq_sb[:],
                        wq.ap().rearrange("(k p) x -> p k x", p=P).bitcast(F32R))

            def load_wv():
                pre_dma(wv_sb[:],
                        wv.ap().rearrange("(k p) o -> p k o", p=P))

            def load_obs():
                # obs streamed in chunks; G accumulates per chunk.  With
                # obs_rot, chunks rotate through a small window instead of a
                # full-obs buffer (2 MB -> cm*obs_rot tiles), freeing SBUF
                # for more nd bufs; each chunk is consumed by its G matmuls
                # right after landing, so a shallow window never stalls.
                for c in range(n_chunks):
                    if obs_rot:
                        och = sc.tile([P, cm, O], F32R, tag="obsch",
                                      bufs=obs_rot, name=f"obsch{c}")
                        obs_tiles.append(och)
                        pre_dma(och[:], obs_re[:, c * cm:(c + 1) * cm, :])
                    else:
                        pre_dma(obs_sb[:, c * cm:(c + 1) * cm, :],
                                obs_re[:, c * cm:(c + 1) * cm, :])

            order = KNOBS.get("prelude_order")
            if order is None:
                order = "owv" if KNOBS["obs_first"] else "wvo"
            def load_nodes_pre():
                for gi in range(KNOBS.get("pre_node_groups", 0)):
                    pre_items.append(load_group(gi * GRP))

            _loaders = {"k": load_wk, "q": load_wq, "v": load_wv,
                        "o": load_obs, "n": load_nodes_pre}
            _seq = {"wvo": "kqvo", "owv": "okqv",
                    "kqov": "kqov", "kqvo": "kqvo",
                    "kvqo": "kvqo", "okvq": "okvq",
                    "koqv": "koqv", "kovq": "kovq", "okqv": "okqv",
                    "nkqvo": "nkqvo", "knqvo": "knqvo",
                    "kqnvo": "kqnvo", "kqvno": "kqvno"}[order]
            for ch in _seq:
                _loaders[ch]()

            # A1T = W_k.T @ W_q  [256, 512] - no obs dependency
            a1t_sb = sc.tile([P, KO, E], F32R)
            for a in range(KO):
                a1_ps = pps.tile([P, E], F32, tag="pps")
                for k in range(KE):
                    nc.tensor.matmul(
                        a1_ps[:], wk_sb[:, k, a * P:(a + 1) * P], wq_sb[:, k, :],
                        start=(k == 0), stop=(k == KE - 1))
                nc.scalar.copy(a1t_sb[:, a, :], a1_ps[:])

            # W_v.T  [256, 512] via PE transpose - no obs dependency
            wvT_sb = sc.tile([P, KO, E], F32R)
            for b in range(KO):
                t_ps = pps.tile([P, E], F32, tag="pps")
                for j in range(KE):
                    nc.tensor.transpose(
                        t_ps[:, j * P:(j + 1) * P],
                        wv_sb[:, j, b * P:(b + 1) * P], ident[:])
                nc.scalar.copy(wvT_sb[:, b, :], t_ps[:])

            # G = obs.T @ obs  [256, 256], accumulated chunk by chunk.
            # WARNING: do NOT move g_ps to its own PSUM tag (or change the
            # warm_ps bufs): that re-tagging compiled and matched the cost
            # model (89267ns) but produced rel-err 0.16 on hardware - the
            # long-lived G accumulation interleaved with other matmul groups
            # appears sensitive to PSUM bank placement.  Keep the shared
            # "pps" ring layout that hardware-validates.
            g_ps = [pps.tile([P, O], F32, tag="pps", name=f"g_ps{a}")
                    for a in range(KO)]
            for c in range(n_chunks):
                for a in range(KO):
                    for t in range(c * cm, (c + 1) * cm):
                        if obs_rot:
                            lhs = obs_tiles[c][:, t - c * cm,
                                               a * P:(a + 1) * P]
                            rhs = obs_tiles[c][:, t - c * cm, :]
                        else:
                            lhs = obs_sb[:, t, a * P:(a + 1) * P]
                            rhs = obs_sb[:, t, :]
                        nc.tensor.matmul(
                            g_ps[a][:], lhs, rhs,
                            start=(t == 0), stop=(t == MT - 1))
            g_sb = sc.tile([P, KO, O], F32R)
            for a in range(KO):
                (nc.vector.tensor_copy if KNOBS.get("pre_copy_alt") and
                 a % 2 else nc.scalar.copy)(g_sb[:, a, :], g_ps[a][:])

            # T1 = G @ W_v.T  [256, 512]  (G symmetric -> G tiles usable as lhsT)
            t1_sb = sc.tile([P, KO, E], F32R)
            for a in range(KO):
                t1_ps = pps.tile([P, E], F32, tag="pps")
                for b in range(KO):
                    nc.tensor.matmul(
                        t1_ps[:], g_sb[:, b, a * P:(a + 1) * P], wvT_sb[:, b, :],
                        start=(b == 0), stop=(b == KO - 1))
                (nc.vector.tensor_copy if KNOBS.get("pre_copy_alt") and
                 a % 2 == 0 else nc.scalar.copy)(t1_sb[:, a, :], t1_ps[:])

            # W_tot = A1 @ T1 + I  [512, 512]
            dve_fold = KNOBS.get("i_fold", "dve") == "dve"
            if dve_fold and KNOBS.get("wtot_bmajor", False):
                # b-major emission: the first KE matmuls need only T1 block 0,
                # hiding T1 block 1's compute+copy latency behind real PE work
                # on the prelude-exit critical chain
                w_pss = [pps.tile([P, E], F32, tag="pps", name=f"w_ps{x}")
                         for x in range(KE)]
                for b in range(KO):
                    for x in range(KE):
                        nc.tensor.matmul(
                            w_pss[x][:], a1t_sb[:, b, x * P:(x + 1) * P],
                            t1_sb[:, b, :], start=(b == 0),
                            stop=(b == KO - 1), skip_group_check=True)
                for x in range(KE):
                    (nc.vector.tensor_copy if KNOBS.get("pre_copy_alt") and
                     x % 2 else nc.scalar.copy)(wtot[:, x, :], w_pss[x][:])
                    nc.vector.tensor_add(
                        wtot[:, x, x * P:(x + 1) * P],
                        wtot[:, x, x * P:(x + 1) * P], ident_bw[:])
            else:
                for x in range(KE):
                    w_ps = pps.tile([P, E], F32, tag="pps")
                    for b in range(KO):
                        nc.tensor.matmul(
                            w_ps[:], a1t_sb[:, b, x * P:(x + 1) * P],
                            t1_sb[:, b, :], start=(b == 0),
                            stop=(dve_fold and b == KO - 1))
                    if not dve_fold:
                        nc.tensor.matmul(
                            w_ps[:], ident_r[:],
                            zsh_r[:, KE * P - x * P: 2 * KE * P - x * P],
                            start=False, stop=True)
                    (nc.vector.tensor_copy if KNOBS.get("pre_copy_alt") and
                     x % 2 else nc.scalar.copy)(wtot[:, x, :], w_ps[:])
                    if dve_fold:
                        nc.vector.tensor_add(
                            wtot[:, x, x * P:(x + 1) * P],
                            wtot[:, x, x * P:(x + 1) * P], ident_bw[:])

            if apply_affine:
                ones_r = sc.tile([1, P], F32R)
                nc.gpsimd.memset(ones_r[:], 1.0)
                gam_sb = sc.tile([1, E], F32R)
                nc.sync.dma_start(gam_sb[:], gam.ap().bitcast(F32R))
                bet_sb = sc.tile([1, E], F32R)
                nc.sync.dma_start(bet_sb[:], bet.ap().bitcast(F32R))
                for (src, dst) in ((gam_sb, gbc), (bet_sb, bbc)):
                    bc_ps = pps.tile([P, E], F32, tag="pps")
                    nc.tensor.matmul(bc_ps[:], ones_r[:], src[:])
                    nc.scalar.copy(dst[:], bc_ps[:])

        # ---------------- main loop over node row tiles ----------------------
        psT_pool = ctx.enter_context(
            tc.tile_pool(name="pstp", bufs=KNOBS["pst_bufs"], space="PSUM"))
        acc_pool = ctx.enter_context(
            tc.tile_pool(name="accp", bufs=KNOBS["acc_bufs"], space="PSUM"))
        _eng = {"scalar": nc.scalar.dma_start,
                "gpsimd": nc.gpsimd.dma_start,
                "sync": nc.sync.dma_start}
        _st_cnt = [0]

        def store_dma(dst, src, tile_idx=None):
            se = KNOBS["store_engine"]
            pr = KNOBS.get("pool_store_range")
            if pr is not None and tile_idx is not None \
                    and pr[0] <= tile_idx < pr[1]:
                _st_cnt[0] += 1
                return nc.gpsimd.dma_start(dst, src)
            if se == "alt":          # alternate ACT / SP queues
                fn = (nc.scalar.dma_start if _st_cnt[0] % 2 == 0
                      else nc.sync.dma_start)
            elif se == "altg":       # alternate ACT / Pool queues
                fn = (nc.scalar.dma_start if _st_cnt[0] % 2 == 0
                      else nc.gpsimd.dma_start)
            elif se == "sg":         # alternate SP / Pool queues
                fn = (nc.sync.dma_start if _st_cnt[0] % 2 == 0
                      else nc.gpsimd.dma_start)
            elif se == "sga":        # rotate SP / Pool / ACT queues
                fn = (nc.sync.dma_start, nc.gpsimd.dma_start,
                      nc.scalar.dma_start)[_st_cnt[0] % 3]
            else:
                fn = _eng[se]
            _st_cnt[0] += 1
            return fn(dst, src)

        node_ap = node.ap()
        out_ap = out.ap()
        GRP = KNOBS["group"]

        def stage_tr(t0g, g):
            """Loads + PE transposes + PSUM->SBUF copies for one group.
            Returns [(ndT, rn, r0), ...] for stage_acc."""
            r0 = t0g * P
            full = (t0g + g < NT) or (LAST == P)
            rows = g * P if full else (g - 1) * P + LAST
            nd = node_pool.tile([P, GRP, E], F32, tag="nd")
            if full and g == GRP:
                nc.sync.dma_start(
                    nd[:], node_ap[r0:r0 + g * P, :].rearrange(
                        "(b p) e -> p b e", p=P))
            else:
                for j in range(g):
                    rn_j = min(P, rows - j * P)
                    nc.sync.dma_start(nd[:rn_j, j, :],
                                      node_ap[r0 + j * P:r0 + j * P + rn_j, :])
            tr_mode = KNOBS["tr_mode"]
            if tr_mode == "bf16":
                # downcast once per group, then 1.0 cyc/row PE transposes
                ndb = ndT_pool.tile([P, GRP, E], BF16, tag="ndb")
                for j in range(g):
                    rn = min(P, rows - j * P)
                    cv = nc.scalar.copy if (t0g + j) % 2 == 0 \
                        else nc.vector.tensor_copy
                    cv(ndb[:rn, j, :], nd[:rn, j, :])
            ps_dt = {"bf16": BF16, "f32r": F32R, "f32": F32}[tr_mode]
            trs = []
            for j in range(g):
                rn = min(P, rows - j * P)
                psT = psT_pool.tile([P, E], ps_dt, tag="psT")
                for k in range(KE):
                    if tr_mode == "bf16":
                        nc.tensor.transpose(
                            psT[:, k * P:k * P + rn],
                            ndb[:rn, j, k * P:(k + 1) * P], ident_b[:rn, :rn])
                    elif tr_mode == "f32r":
                        nc.tensor.transpose(
                            psT[:, k * P:k * P + rn],
                            nd[:rn, j, k * P:(k + 1) * P].bitcast(F32R),
                            ident_fr[:rn, :rn])
                    else:
                        nc.tensor.transpose(
                            psT[:, k * P:k * P + rn],
                            nd[:rn, j, k * P:(k + 1) * P], ident[:rn, :rn])
                ndT = ndT_pool.tile([P, E], BF16, tag="ndT")
                t = t0g + j
                cp = nc.scalar.copy if t % 2 == 0 else nc.vector.tensor_copy
                ps_src = psT.bitcast(F32) if tr_mode == "f32r" else psT
                if rn == P:
                    cp(ndT[:], ps_src[:])
                else:
                    for k in range(KE):
                        cp(ndT[:, k * P:k * P + rn],
                           ps_src[:, k * P:k * P + rn])
                trs.append((ndT, rn, r0 + j * P))
            return trs

        def stage_acc(trs):
            """Accumulating matmuls for one group. Returns [(acc, rn, r0)]."""
            accs = []
            for ndT, rn, r0 in trs:
                acc = acc_pool.tile([P, E], F32, tag="acc")
                for k in range(KE):
                    nc.tensor.matmul(
                        acc[:rn, :], ndT[:, k * P:k * P + rn], wtot[:, k, :],
                        start=(k == 0), stop=(k == KE - 1))
                accs.append((acc, rn, r0))
            return accs

        def stage_mm(t0g, g):
            """Original per-tile interleaved emission: tr a, copy a, mm a,
            tr b, copy b, mm b — measurably better for the PE pipeline than
            batching all transposes before all matmuls."""
            r0 = t0g * P
            full = (t0g + g < NT) or (LAST == P)
            rows = g * P if full else (g - 1) * P + LAST
            nd = node_pool.tile([P, GRP, E], F32, tag="nd")
            if full and g == GRP:
                nc.sync.dma_start(
                    nd[:], node_ap[r0:r0 + g * P, :].rearrange(
                        "(b p) e -> p b e", p=P))
            else:
                for j in range(g):
                    rn_j = min(P, rows - j * P)
                    nc.sync.dma_start(nd[:rn_j, j, :],
                                      node_ap[r0 + j * P:r0 + j * P + rn_j, :])
            tr_mode = KNOBS["tr_mode"]
            ps_dt = {"bf16": BF16, "f32r": F32R, "f32": F32}[tr_mode]
            accs = []
            for j in range(g):
                rn = min(P, rows - j * P)
                psT = psT_pool.tile([P, E], ps_dt, tag="psT")
                for k in range(KE):
                    nc.tensor.transpose(
                        psT[:, k * P:k * P + rn],
                        nd[:rn, j, k * P:(k + 1) * P], ident[:rn, :rn])
                ndT = ndT_pool.tile([P, E], BF16, tag="ndT")
                t = t0g + j
                cc = KNOBS.get("copy_chunks", 1)
                if rn == P and cc == 1:
                    cp = (nc.scalar.copy if t % 2 == 0
                          else nc.vector.tensor_copy)
                    cp(ndT[:], psT[:])
                elif rn == P:
                    # chunked bounce copy: mm k can start after chunk k lands
                    w = E // cc
                    for c in range(cc):
                        cp = (nc.scalar.copy if (t + c) % 2 == 0
                              else nc.vector.tensor_copy)
                        cp(ndT[:, c * w:(c + 1) * w], psT[:, c * w:(c + 1) * w])
                else:
                    cp = (nc.scalar.copy if t % 2 == 0
                          else nc.vector.tensor_copy)
                    for k in range(KE):
                        cp(ndT[:, k * P:k * P + rn], psT[:, k * P:k * P + rn])
                acc = acc_pool.tile([P, E], F32, tag="acc")
                for k in range(KE):
                    nc.tensor.matmul(
                        acc[:rn, :], ndT[:, k * P:k * P + rn], wtot[:, k, :],
                        start=(k == 0), stop=(k == KE - 1))
                accs.append((acc, rn, r0 + j * P))
            return accs

        def stage_ln(accs):
            """LayerNorm for a group; sqrt/recip batched across the group.
            Returns [(ot, rn, r0), ...] for the store stage."""
            g = len(accs)
            mv = stat_pool.tile([P, g, 2], F32, tag="mv")
            for j, (acc, rn, _) in enumerate(accs):
                bn6 = stat_pool.tile([P, 6], F32, tag="bn6")
                nc.vector.bn_stats(bn6[:rn], acc[:rn, :])
                nc.vector.bn_aggr(mv[:rn, j, :], bn6[:rn])
            rnmax = max(rn for _, rn, _ in accs)
            std = stat_pool.tile([P, g], F32, tag="std")
            nc.scalar.activation(std[:rnmax], mv[:rnmax, :, 1], AF.Sqrt,
                                 bias=eps_t[:rnmax], scale=1.0)
            rstd = stat_pool.tile([P, g], F32, tag="rstd")
            nc.vector.reciprocal(rstd[:rnmax], std[:rnmax])
            grp_store = KNOBS["store_grp"]
            otg = (out_pool.tile([P, g, E], F32, tag="otg", name="otg")
                   if grp_store else None)
            outs = []
            for j, (acc, rn, r0) in enumerate(accs):
                nmr = stat_pool.tile([P, 1], F32, tag="nmr")  # -mean * rstd
                npf = KNOBS.get("nmr_pool_from")
                nmr_eng = nc.vector
                if npf is not None and r0 // P >= npf:
                    # all-SBUF op: legal on the idle Pool engine; keeps the
                    # final tiles' nmr from queueing behind a DVE normalize
                    nmr_eng = nc.gpsimd
                nmr_eng.tensor_scalar(nmr[:rn], mv[:rn, j, 0:1],
                                      rstd[:rn, j:j + 1], -1.0,
                                      ALU.mult, ALU.mult)
                if grp_store:
                    ot, ot_ap = None, otg[:rn, j, :]
                else:
                    ot = out_pool.tile([P, E], F32, tag="ot")
                    ot_ap = ot[:rn, :]
                naf = KNOBS.get("norm_alt_from")
                npf = KNOBS.get("norm_pool_from")
                tix = r0 // P
                if npf is not None and tix >= npf and tix % 3 == 2:
                    nc.gpsimd.tensor_scalar(ot_ap, acc[:rn, :],
                                            rstd[:rn, j:j + 1], nmr[:rn],
                                            ALU.mult, ALU.add)
                elif (KNOBS["norm_alt"] or (naf is not None and tix >= naf)) \
                        and tix % 2 == 1:
                    nc.vector.tensor_scalar(ot_ap, acc[:rn, :],
                                            rstd[:rn, j:j + 1], nmr[:rn],
                                            ALU.mult, ALU.add)
                else:
                    nc.scalar.activation(ot_ap, acc[:rn, :], AF.Identity,
                                         bias=nmr[:rn], scale=rstd[:rn, j:j + 1])
                if apply_affine:
                    nc.vector.tensor_mul(ot_ap, ot_ap, gbc[:rn])
                    nc.vector.tensor_add(ot_ap, ot_ap, bbc[:rn])
                outs.append((ot, rn, r0))
            return (otg, outs)

        def stage_store(packed):
            otg, outs = packed
            rows = sum(rn for _, rn, _ in outs)
            g = len(outs)
            if otg is not None and g > 1 and rows == g * P:
                # single grouped DMA: one dispatch + one HWDGE slot per group
                r0 = outs[0][2]
                store_dma(
                    out_ap[r0:r0 + g * P, :].rearrange("(b p) e -> p b e", p=P),
                    otg[:, :, :])
                return
            for j, (ot, rn, r0) in enumerate(outs):
                src = otg[:rn, j, :] if otg is not None else ot[:rn, :]
                store_dma(out_ap[r0:r0 + rn, :], src, tile_idx=r0 // P)


        def stage_tr3(item):
            """PE transposes + PSUM->SBUF copies for one pre-loaded group."""
            t0g, g, nd, rows = item
            tr_mode = KNOBS["tr_mode"]
            ttf = KNOBS.get("tr_tail_from")
            if ttf is not None and t0g >= ttf:
                tr_mode = KNOBS.get("tr_tail_mode", "f32r")
            ps_dt = {"bf16": BF16, "f32r": F32R, "f32": F32}[tr_mode]
            if tr_mode == "bf16":
                ndb = ndT_pool.tile([P, GRP, E], BF16, tag="ndb",
                                    bufs=KNOBS.get("ndb_bufs", 4))
                for j in range(g):
                    rn = min(P, rows - j * P)
                    nde = KNOBS.get("ndb_engine")
                    if nde == "gpsimd":
                        # SBUF->SBUF downcast is legal on the (idle) Pool
                        # engine, unlike the PSUM-touching bounce copies
                        cv = nc.gpsimd.tensor_copy
                    else:
                        cv = nc.scalar.copy if (t0g + j) % 2 == 0 \
                            else nc.vector.tensor_copy
                    cv(ndb[:rn, j, :], nd[:rn, j, :])
            trs = []
            for j in range(g):
                rn = min(P, rows - j * P)
                psT = psT_pool.tile([P, E], ps_dt, tag="psT")
                for k in range(KE):
                    if tr_mode == "f32r":
                        nc.tensor.transpose(
                            psT[:, k * P:k * P + rn],
                            nd[:rn, j, k * P:(k + 1) * P],
                            ident_fr[:rn, :rn])
                    elif tr_mode == "bf16":
                        nc.tensor.transpose(
                            psT[:, k * P:k * P + rn],
                            ndb[:rn, j, k * P:(k + 1) * P], ident_b[:rn, :rn])
                    else:
                        nc.tensor.transpose(
                            psT[:, k * P:k * P + rn],
                            nd[:rn, j, k * P:(k + 1) * P], ident[:rn, :rn])
                ndT = ndT_pool.tile([P, E], BF16, tag="ndT")
                t = t0g + j
                # NOTE: gpsimd cannot be used for these copies - the source
                # psT is PSUM and GPSIMD instructions cannot access PSUM
                # (BIR verifier rejects it; the cost model doesn't know).
                if True:
                    par = t % 2
                    cpf = KNOBS.get("copy_flip_from")
                    if cpf is not None and t >= cpf:
                        par ^= 1
                    if t < KNOBS.get("early_copy_dve_until", 0):
                        par = 1      # keep ACT free for the W_tot exit chain
                    cp = nc.scalar.copy if par == 0 else nc.vector.tensor_copy
                ps_src = psT.bitcast(F32) if tr_mode == "f32r" else psT
                if rn == P:
                    cp(ndT[:], ps_src[:])
                else:
                    for k in range(KE):
                        cp(ndT[:, k * P:k * P + rn],
                           ps_src[:, k * P:k * P + rn])
                trs.append((ndT, rn, t0g * P + j * P))
            return trs

        # software pipeline: tr(g+lead) runs ahead of acc(g) so the PSUM->SBUF
        # copy lands before the matmul's Ldweights reaches the PE queue head;
        # LN one group behind acc; stores lag so their data-ready waits never
        # block later dispatches on the store queue.
        LAG = KNOBS.get("store_lag", 1)
        TRL = KNOBS.get("tr_lead", 0)
        if KNOBS["sched"] == "loadfirst":
            # All node loads emitted first: the Tile scheduler (priority =
            # program order) then serves every load before any store on the
            # shared DMA engines, so the DMA device never starves waiting on
            # compute in the tail.  Stores ride the SP queue, which is idle
            # once loads finish.  Transposes lead matmuls by TRL groups so the
            # PE queue never head-blocks on a PSUM->SBUF copy.
            from collections import deque
            grp1_from = KNOBS.get("tail_grp1_from")
            if KNOBS.get("load_lead"):
                # Interleaved emission: the SP queue carries
                # [L0..L(lead), S0,L(lead+1), S1,L(lead+2), ...] so the load
                # stream extends late into the run and ready stores keep the
                # DMA device fed between loads.
                LEADL = KNOBS["load_lead"]
                bounds = []
                t0 = 0
                while t0 < NT:
                    gmax = 1 if (grp1_from is not None and t0 >= grp1_from) \
                        else GRP
                    bounds.append((t0, gmax))
                    t0 += min(gmax, NT - t0)
                pend_loads = deque(
                    load_group(*b) for b in bounds[:LEADL])
                nxt = LEADL
                pend_acc = deque()
                pend_ln = deque()
                pend_st = deque()
                TRL3 = max(1, TRL)
                for i in range(len(bounds)):
                    pend_acc.append(stage_tr3(pend_loads.popleft()))
                    if len(pend_acc) > TRL3:
                        pend_ln.append(stage_acc(pend_acc.popleft()))
                    if len(pend_ln) > 1:
                        pend_st.append(stage_ln(pend_ln.popleft()))
                    if len(pend_st) >= LAG:
                        stage_store(pend_st.popleft())
                    if nxt < len(bounds):
                        pend_loads.append(load_group(*bounds[nxt]))
                        nxt += 1
                while pend_acc:
                    pend_ln.append(stage_acc(pend_acc.popleft()))
                    if len(pend_ln) > 1:
                        pend_st.append(stage_ln(pend_ln.popleft()))
                while pend_ln:
                    pend_st.append(stage_ln(pend_ln.popleft()))
                while pend_st:
                    stage_store(pend_st.popleft())
            items = list(pre_items) if not KNOBS.get("load_lead") else None
            t0 = sum(it[1] for it in pre_items)
            if KNOBS.get("load_lead"):
                pass             # handled above
            elif KNOBS.get("singles_first") and grp1_from is not None \
                    and KNOBS.get("nd1_bufs"):
                # The tail singles' loads are emitted FIRST: their private
                # nd1 slots never recycle, so the data parks in SBUF until
                # their (last-in-pipeline) transposes.  This frees them from
                # the DMA-ring latency chain that otherwise gates the final
                # loads ~2us apart and idles the DMA device.
                k = KNOBS.get("singles_after", 0)
                while t0 < min(grp1_from, k * GRP):
                    items.append(load_group(t0, GRP))
                    t0 += items[-1][1]
                singles = []
                ts = grp1_from
                while ts < NT:
                    singles.append(load_group(ts, 1))
                    ts += 1
                while t0 < grp1_from:
                    items.append(load_group(t0, GRP))
                    t0 += items[-1][1]
                items.extend(singles)
            else:
                while t0 < NT:
                    gmax = 1 if (grp1_from is not None and t0 >= grp1_from) \
                        else GRP
                    items.append(load_group(t0, gmax))
                    t0 += items[-1][1]
            if items is None:
                items = []       # load_lead path already emitted everything
            TRL3 = max(1, TRL)
            trl_tail_from = KNOBS.get("trl_tail_from")
            trl_tail = KNOBS.get("trl_tail", 1)
            pend_acc = deque()
            pend_ln = deque()
            pend_st = deque()
            # Optionally run the tail singles' transposes early, in the
            # mid-phase where PE is input-limited and has slack; the tail
            # then runs matmul+LN only, shrinking the end-of-run compute
            # deficit.  Their ndT tiles park in SBUF until their matmuls.
            EST = KNOBS.get("early_single_tr_at")
            banked_trs = {}
            for item in items:
                t0g = item[0]
                lead = TRL3
                if trl_tail_from is not None and t0g >= trl_tail_from:
                    lead = trl_tail
                if t0g in banked_trs:
                    pend_acc.append(banked_trs.pop(t0g))
                else:
                    pend_acc.append(stage_tr3(item))
                if EST is not None and t0g == EST and grp1_from is not None:
                    for it2 in items:
                        if it2[0] >= grp1_from:
                            banked_trs[it2[0]] = stage_tr3(it2)
                while len(pend_acc) > lead:
                    pend_ln.append(stage_acc(pend_acc.popleft()))
                while len(pend_ln) > 1:
                    pend_st.append(stage_ln(pend_ln.popleft()))
                if len(pend_st) >= LAG:
                    stage_store(pend_st.popleft())
            while pend_acc:
                pend_ln.append(stage_acc(pend_acc.popleft()))
                if len(pend_ln) > 1:
                    pend_st.append(stage_ln(pend_ln.popleft()))
            while pend_ln:
                pend_st.append(stage_ln(pend_ln.popleft()))
            while pend_st:
                stage_store(pend_st.popleft())
        else:
            _emit_interleave(stage_tr, stage_acc, stage_mm, stage_ln,
                             stage_store, TRL, LAG, GRP)

    _legalize_matmult_waits(nc)
    nc.compile()
    return nc


def _legalize_matmult_waits(nc):
    """Split multi-wait Matmults: HW allows 1 sync wait on a matmult (plus 1
    on its Ldweights); the loadfirst schedule's pacing sems can exceed that.
    Hoist ALL waits onto InstEventSemaphore instructions (2 waits each)
    inserted just before the matmult on the same in-order queue.

    Waits on semaphores updated ONLY by same-engine (PE) instructions are
    dropped outright: the PE queue executes in order, so any such update
    precedes this instruction's execution; these waits are scheduler pacing,
    not data dependencies."""
    pe_only_sems = {}
    dma_sems = set()
    for blk in nc.m.functions[0].blocks:
        for inst in blk.instructions:
            si = inst.sync_info
            if si is None:
                continue
            for u in si.on_update:
                key = (str(u.sync_type), u.id)
                if inst.engine != mybir.EngineType.PE:
                    pe_only_sems[key] = False
                else:
                    pe_only_sems.setdefault(key, True)
                if inst.opcode == "DMACopy":
                    dma_sems.add(key)
    for blk in nc.m.functions[0].blocks:
        insts = blk.instructions
        idx = 0
        while idx < len(insts):
            inst = insts[idx]
            si = inst.sync_info
            if si is not None and si.on_wait \
                    and inst.opcode == "DMACopy" \
                    and KNOBS.get("drop_nd1_waits", True):
                # nd1 single loads write private never-reused slots: no WAR
                # hazard exists, so any non-DMA wait is scheduler pacing
                try:
                    is_nd1 = str(inst.outs[0].memref).startswith("nd1")
                except Exception:
                    is_nd1 = False
                if is_nd1:
                    kept = [w for w in si.on_wait
                            if (str(w.sync_type), w.id) in dma_sems]
                    if len(kept) != len(si.on_wait):
                        inst.sync_info = mybir.SyncInfo(
                            on_wait=kept, on_update=si.on_update)
                        si = inst.sync_info
            if si is not None and si.on_wait \
                    and inst.opcode == "DMACopy" \
                    and KNOBS.get("drop_store_pe_waits", True):
                # an output store never depends on PE directly (LN on ACT/DVE
                # produces its data); PE-sem waits on stores are scheduler
                # pacing only
                try:
                    is_store = inst.outs[0].memref == "out"
                except Exception:
                    is_store = False
                if is_store:
                    kept = [w for w in si.on_wait
                            if not pe_only_sems.get(
                                (str(w.sync_type), w.id), False)]
                    if len(kept) != len(si.on_wait):
                        inst.sync_info = mybir.SyncInfo(
                            on_wait=kept, on_update=si.on_update)
                        si = inst.sync_info
            if si is not None and si.on_wait \
                    and inst.engine == mybir.EngineType.PE \
                    and KNOBS.get("drop_pe_waits", True):
                kept = [w for w in si.on_wait
                        if not pe_only_sems.get(
                            (str(w.sync_type), w.id), False)]
                if len(kept) != len(si.on_wait):
                    inst.sync_info = mybir.SyncInfo(on_wait=kept,
                                                    on_update=si.on_update)
                    si = inst.sync_info
            if inst.opcode == "Matmult" and si is not None \
                    and len(si.on_wait) > 1:
                waits = list(si.on_wait)
                # keep a DMA-sem wait on the matmult if present (it is the
                # late-arriving one); relocate the rest
                keep_i = next(
                    (i for i, w in enumerate(waits)
                     if (str(w.sync_type), w.id) in dma_sems),
                    len(waits) - 1)
                keep = [waits.pop(keep_i)]
                inst.sync_info = mybir.SyncInfo(on_wait=keep,
                                                on_update=si.on_update)
                pos = idx
                for c in range(0, len(waits), 2):
                    ev = mybir.InstEventSemaphore(
                        name=nc.get_next_instruction_name(),
                        engine=inst.engine,
                        ins=[], outs=[],
                        sync_info=mybir.SyncInfo(
                            on_wait=waits[c:c + 2], on_update=[]))
                    nc.register_instruction(ev)
                    insts.insert(pos, ev)
                    pos += 1
                idx = pos
            idx += 1


def _emit_interleave(stage_tr, stage_acc, stage_mm, stage_ln, stage_store,
                     TRL, LAG, GRP):
    for _rep in range(KNOBS.get("repeat", 1)):
            from collections import deque
            pend_acc = deque()  # transposed groups awaiting matmuls
            pend_ln = deque()   # acc'd groups awaiting LN emission
            pend_st = deque()   # LN'd groups awaiting store emission
            t = 0
            while t < NT:
                g = min(GRP, NT - t)
                if TRL == 0:
                    pend_ln.append(stage_mm(t, g))
                else:
                    pend_acc.append(stage_tr(t, g))
                    if len(pend_acc) > TRL:
                        pend_ln.append(stage_acc(pend_acc.popleft()))
                if len(pend_ln) > 1:
                    pend_st.append(stage_ln(pend_ln.popleft()))
                if len(pend_st) >= LAG:
                    stage_store(pend_st.popleft())
                t += g
            while pend_acc:
                pend_ln.append(stage_acc(pend_acc.popleft()))
                if len(pend_ln) > 1:
                    pend_st.append(stage_ln(pend_ln.popleft()))
            while pend_ln:
                pend_st.append(stage_ln(pend_ln.popleft()))
            while pend_st:
                stage_store(pend_st.popleft())

    nc.compile()
    return nc


_CACHE: dict = {}


def _get_runner(apply_affine: bool):
    """Build + jit once; returns a dict with jitted runners + io metadata."""
    key = apply_affine
    if key in _CACHE:
        return _CACHE[key]

    import jax
    from jax.sharding import Mesh, PartitionSpec
    from jax.experimental.shard_map import shard_map
    from concourse import bass2jax

    nc = _build(apply_affine)
    bass2jax.install_neuronx_cc_hook()

    partition_name = (nc.partition_id_tensor.name
                      if nc.partition_id_tensor else None)
    in_names, out_names, out_avals, zero_outs = [], [], [], []
    for alloc in nc.m.functions[0].allocations:
        if not isinstance(alloc, mybir.MemoryLocationSet):
            continue
        name = alloc.memorylocations[0].name
        if alloc.kind == "ExternalInput":
            if name != partition_name:
                in_names.append(name)
        elif alloc.kind == "ExternalOutput":
            shape = tuple(alloc.tensor_shape)
            dtype = mybir.dt.np(alloc.dtype)
            out_names.append(name)
            out_avals.append(jax.core.ShapedArray(shape, dtype))
            zero_outs.append(np.zeros(shape, dtype))
    n_params = len(in_names)
    all_names = in_names + out_names
    if partition_name is not None:
        all_names = all_names + [partition_name]
    donate = tuple(range(n_params, n_params + len(out_names)))

    def _body(*args):
        operands = list(args)
        if partition_name is not None:
            operands.append(bass2jax.partition_id_tensor())
        outs = bass2jax._bass_exec_p.bind(
            *operands,
            out_avals=tuple(out_avals),
            in_names=tuple(all_names),
            out_names=tuple(out_names),
            lowering_input_output_aliases=(),
            sim_require_finite=True,
            sim_require_nnan=True,
            nc=nc,
        )
        return tuple(outs)

    devices = jax.devices()[:N_CORES]
    mesh = Mesh(np.asarray(devices), ("core",))
    n_io = n_params + len(out_names)
    mapped = shard_map(_body, mesh=mesh,
                       in_specs=(PartitionSpec("core"),) * n_io,
                       out_specs=(PartitionSpec("core"),) * len(out_names))
    sharded = jax.jit(mapped, donate_argnums=donate, keep_unused=True)
    sharded_t = jax.jit(mapped, keep_unused=True)  # non-donating, reusable args

    shardings = [jax.sharding.NamedSharding(mesh, PartitionSpec("core"))] * n_io

    def put(arrs):
        return [jax.device_put(a, s) for a, s in zip(arrs, shardings)]

    runner = {
        "fn": sharded,
        "fn_t": sharded_t,
        "put": put,
        "in_names": in_names,
        "out_names": out_names,
        "zero_outs": zero_outs,
    }
    _CACHE[key] = runner
    return runner


def _prep_inputs(runner, inputs_np: dict) -> list:
    """Concat per-core inputs along axis 0 (global arrays for shard_map)."""
    concat = []
    for name in runner["in_names"]:
        per_core = inputs_np[name]           # list of 8 per-core arrays
        concat.append(np.concatenate(per_core, axis=0))
    for z in runner["zero_outs"]:
        concat.append(np.zeros((N_CORES * z.shape[0], *z.shape[1:]), z.dtype))
    return concat


def _make_per_core(node_feature, obs_feature, W_q, W_k, W_v, gam, bet,
                   apply_affine):
    f = np.ascontiguousarray
    per = {
        "node": [f(node_feature[c * NP:(c + 1) * NP]) for c in range(N_CORES)],
        "obs": [f(obs_feature)] * N_CORES,
        "wq": [f(W_q)] * N_CORES,
        "wk": [f(W_k)] * N_CORES,
        "wv": [f(W_v)] * N_CORES,
    }
    if apply_affine:
        per["gam"] = [f(gam.reshape(1, E))] * N_CORES
        per["bet"] = [f(bet.reshape(1, E))] * N_CORES
    return per


def kernel(node_feature, obs_feature, W_q, W_k, W_v, ln_gamma, ln_beta):
    node_feature = np.asarray(node_feature, dtype=np.float32)
    obs_feature = np.asarray(obs_feature, dtype=np.float32)
    W_q = np.asarray(W_q, dtype=np.float32)
    W_k = np.asarray(W_k, dtype=np.float32)
    W_v = np.asarray(W_v, dtype=np.float32)
    ln_gamma = np.asarray(ln_gamma, dtype=np.float32)
    ln_beta = np.asarray(ln_beta, dtype=np.float32)

    apply_affine = not (np.all(ln_gamma == 1.0) and np.all(ln_beta == 0.0))
    runner = _get_runner(apply_affine)
    per = _make_per_core(node_feature, obs_feature, W_q, W_k, W_v,
                         ln_gamma, ln_beta, apply_affine)
    args = _prep_inputs(runner, per)
    outs = runner["fn"](*args)
    res = np.asarray(outs[runner["out_names"].index("out")])
    return res.reshape(N_TOT, E)

